# revision 24
# baseline (speedup 1.0000x reference)
"""Trainium2 Bass kernel for nn_CostVolume: H-sharded across 8 NeuronCores.

v2: bf16 matmuls + M=128 pair-packing for conv3b + narrowed compute.

- BN folded into conv weights on host; all matmul operands bf16 (end-to-end
  rel err ~5e-3 vs fp32 reference, tolerance 2e-2).
- down(): 1x1 conv K=1024; x shipped bf16, loaded as 8 big DMAs per side.
- conv3a collapsed into G_L/G_R/E/F 2D convs (cost-volume shift structure);
  K-packed over (kh0,kh2) via row-shifted stacked tiles Lp2/Rp2, M-packed
  over variant pairs. b3a baked into G_L evac.
- A[d] assembled per-d by DVE (add/sub/relu) into paired tiles
  C_j = [A[2j]; A[2j+1]]; U_j = [A[2j-1]; A[2j+2]] built by 2 half-copies.
- conv3b M=128 pair-packed: outputs (2j, 2j+1) computed together, 18 matmuls
  per (pair, 3-row chunk) instead of 30: 9 taps on C_j ([w1;w0],[w2;w1]) and
  9 on U_j ([w0;0],[0;w2]).
- w < d-4 output region is constant (masked cost volume): skipped on-device,
  pre-filled host-side in the initial y buffer; A tiles rely on ring-stale
  columns matching the same constant.
"""

import sys

sys.path.insert(0, "/opt/trn_rl_repo")

import numpy as np
import ml_dtypes
import concourse.bass as bass
import concourse.bacc as bacc
import concourse.mybir as mybir
from concourse import tile

F32 = mybir.dt.float32
BF = mybir.dt.bfloat16
I32 = mybir.dt.int32
RELU = mybir.ActivationFunctionType.Relu
IDENT = mybir.ActivationFunctionType.Identity

H, W, D, CF, CIN = 48, 160, 48, 64, 1024
EPS = 1e-5
NC = 8
HLOC = 6
ROWS_IN = 10
ROWS_A = 8
WP = 162
GVLO = -10  # Gr col range [v=-10, 160)
GW = 170
NRING = 4
EV0 = 112
EW = 48
FW = 52
NEG = -1.0e30
S5 = (-2, -1, 0, 1, 2)

KDSETS = {0: (0, 1, 2), 1: (1, 2), 2: (0, 1)}


def _var(d):
    return 1 if d == 0 else (2 if d == D - 1 else 0)


def _fold_bn(w, b, g, beta, m, v):
    s = (g / np.sqrt(v + EPS)).astype(np.float32)
    return (w * s.reshape(-1, *([1] * (w.ndim - 1)))).astype(np.float32), (
        (b - m) * s + beta
    ).astype(np.float32)


def _f_combos():
    combos = []
    for var, kds in KDSETS.items():
        urange = (0, 1) if var == 1 else (-2, -1, 0, 1)
        for u in urange:
            kws = [kw for kw in range(3) if any(kd > u + kw for kd in kds)]
            if kws:
                combos.append((var, u, kws))
    return combos


F_COMBOS = _f_combos()
FIDX = {(var, u): fi for fi, (var, u, _) in enumerate(F_COMBOS)}
# M-pack groups for F: (top fi, bottom fi), kw list per group
FGROUPS = [(0, 1), (2, 7), (4, 6), (5, 3), (None, 8)]
FKWL = [[0, 1, 2], [0, 1], [0, 1, 2], [0], [0]]
FQ = [(g, kw) for g, kws in enumerate(FKWL) for kw in kws]  # 10 flat taps
NFQ = len(FQ)


def _bcast0(ap, n):
    return bass.AP(ap.tensor, ap.offset, list(ap.ap) + [[0, n]])


def _bf(x):
    return np.asarray(x, np.float32).astype(ml_dtypes.bfloat16)


def build_nc():
    nc = bacc.Bacc("TRN2", target_bir_lowering=False, debug=False, num_devices=NC)

    xl_d = nc.dram_tensor("xl", [CIN, ROWS_IN * W], BF, kind="ExternalInput")
    xr_d = nc.dram_tensor("xr", [CIN, ROWS_IN * W], BF, kind="ExternalInput")
    w1t_d = nc.dram_tensor("w1t", [128, 8, 128], BF, kind="ExternalInput")
    wgl0_d = nc.dram_tensor("wgl0", [128, 3, 128], BF, kind="ExternalInput")
    wgl0b_d = nc.dram_tensor("wgl0b", [64, 3, 128], BF, kind="ExternalInput")
    wgl12_d = nc.dram_tensor("wgl12", [128, 3, 128], BF, kind="ExternalInput")
    wgl12b_d = nc.dram_tensor("wgl12b", [64, 3, 128], BF, kind="ExternalInput")
    wgr0_d = nc.dram_tensor("wgr0", [128, 5, 128], BF, kind="ExternalInput")
    wgr0b_d = nc.dram_tensor("wgr0b", [64, 5, 128], BF, kind="ExternalInput")
    wgr12_d = nc.dram_tensor("wgr12", [128, 5, 128], BF, kind="ExternalInput")
    wgr12b_d = nc.dram_tensor("wgr12b", [64, 5, 128], BF, kind="ExternalInput")
    we0_d = nc.dram_tensor("we0", [128, 3, 128], BF, kind="ExternalInput")
    we0b_d = nc.dram_tensor("we0b", [64, 3, 128], BF, kind="ExternalInput")
    we12_d = nc.dram_tensor("we12", [128, 3, 128], BF, kind="ExternalInput")
    we12b_d = nc.dram_tensor("we12b", [64, 3, 128], BF, kind="ExternalInput")
    wf_d = nc.dram_tensor("wf", [128, NFQ, 128], BF, kind="ExternalInput")
    wfb_d = nc.dram_tensor("wfb", [64, NFQ, 128], BF, kind="ExternalInput")
    wct_d = nc.dram_tensor("wct", [128, 9, 128], BF, kind="ExternalInput")
    wut_d = nc.dram_tensor("wut", [128, 9, 128], BF, kind="ExternalInput")
    b1c_d = nc.dram_tensor("b1c", [128, 1], F32, kind="ExternalInput")
    b1r_d = nc.dram_tensor("b1r", [1, 128], BF, kind="ExternalInput")
    rmw_d = nc.dram_tensor("rmw", [1, ROWS_IN, W], BF, kind="ExternalInput")
    b3a2_d = nc.dram_tensor("b3a2", [128, 1], F32, kind="ExternalInput")
    b3b2_d = nc.dram_tensor("b3b2", [128, 1], F32, kind="ExternalInput")
    rowm_d = nc.dram_tensor("rowm", [128, ROWS_IN], BF, kind="ExternalInput")
    grm_d = nc.dram_tensor("grm", [128, ROWS_A], F32, kind="ExternalInput")
    y_d = nc.dram_tensor("y", [D, CF, HLOC, W], F32, kind="ExternalOutput")

    with tile.TileContext(nc) as tc:
        with (
            tc.tile_pool(name="wpool", bufs=1) as wpool,
            tc.tile_pool(name="big", bufs=1) as big,
            tc.tile_pool(name="ost", bufs=4) as ostp,
            tc.tile_pool(name="psd", bufs=2, space="PSUM") as psd_p,
            tc.tile_pool(name="psg", bufs=2, space="PSUM") as psg_p,
            tc.tile_pool(name="ps3", bufs=4, space="PSUM") as ps3_p,
        ):
            # ---- persistent tiles ----
            xt = [big.tile([128, 8, ROWS_IN, W], BF, name=f"xt{s}") for s in range(2)]

            # ---- input/weight DMAs, ordered for startup overlap ----
            def xload(side, x_d, r0, r1):
                # one DMA covering all 8 K-chunks for rows [r0, r1)
                nc.sync.dma_start(
                    xt[side][:, :, r0:r1, :],
                    x_d[:, W * r0 : W * r1].rearrange(
                        "(k p) (r c) -> p k r c", p=128, r=r1 - r0
                    ),
                )

            def wt(dram, shape):
                t = wpool.tile(shape, BF, name=dram.name + "_t")
                nc.sync.dma_start(t[:], dram[:])
                return t

            w1t = wt(w1t_d, [128, 8, 128])
            b1r = wt(b1r_d, [1, 128])
            rmw = wt(rmw_d, [1, ROWS_IN, W])
            xload(0, xl_d, 0, 2)
            for r0 in (2, 4, 6, 8):
                xload(0, xl_d, r0, r0 + 2)
            for r0 in (0, 2, 4, 6, 8):
                xload(1, xr_d, r0, r0 + 2)
            wgl0 = wt(wgl0_d, [128, 3, 128])
            wgl0b = wt(wgl0b_d, [64, 3, 128])
            wgl12 = wt(wgl12_d, [128, 3, 128])
            wgl12b = wt(wgl12b_d, [64, 3, 128])
            rowm = wpool.tile([128, ROWS_IN], BF)
            nc.sync.dma_start(rowm[:], rowm_d[:])
            b3a2 = wpool.tile([128, 1], F32)
            nc.sync.dma_start(b3a2[:], b3a2_d[:])
            wf = wt(wf_d, [128, NFQ, 128])
            wfb = wt(wfb_d, [64, NFQ, 128])
            wgr0 = wt(wgr0_d, [128, 5, 128])
            wgr0b = wt(wgr0b_d, [64, 5, 128])
            wgr12 = wt(wgr12_d, [128, 5, 128])
            wgr12b = wt(wgr12b_d, [64, 5, 128])
            grm = wpool.tile([128, ROWS_A], F32)
            nc.sync.dma_start(grm[:], grm_d[:])
            we0 = wt(we0_d, [128, 3, 128])
            we0b = wt(we0b_d, [64, 3, 128])
            we12 = wt(we12_d, [128, 3, 128])
            we12b = wt(we12b_d, [64, 3, 128])
            wct = wt(wct_d, [128, 9, 128])
            wut = wt(wut_d, [128, 9, 128])
            b3b2 = wpool.tile([128, 1], F32)
            nc.sync.dma_start(b3b2[:], b3b2_d[:])
            Lp2 = big.tile([128, ROWS_IN, WP], BF)
            Rp2 = big.tile([128, ROWS_IN, 212], BF)
            Gld = big.tile([128, ROWS_A, W], BF)  # var0 both halves
            Gle = big.tile([128, ROWS_A, W], BF)  # top var1, bottom var2
            Grd = big.tile([128, ROWS_A, GW], BF)
            Gre = big.tile([128, ROWS_A, GW], BF)
            Etd = big.tile([128, ROWS_A, EW], BF)
            Ete = big.tile([128, ROWS_A, EW], BF)
            Fts = [big.tile([128, ROWS_A, FW], BF, name=f"Ft{i}") for i in range(4)]
            FE = [big.tile([128, ROWS_A, FW], BF, name=f"FE{i}") for i in range(3)]
            Cring = [
                big.tile([128, ROWS_A, WP], BF, name=f"C{i}") for i in range(NRING)
            ]
            Uring = [
                big.tile([128, ROWS_A, WP], BF, name=f"U{i}") for i in range(NRING)
            ]

            # warm the ACT function table while DMAs stream
            scr = wpool.tile([1, 2], F32, name="scr")
            nc.vector.memset(scr[:], 0)
            nc.scalar.activation(scr[:], scr[:], RELU)

            # pad memsets (before writes)
            nc.vector.memset(Lp2[0:64, :, 0:1], 0)
            nc.vector.memset(Lp2[0:64, :, 161:162], 0)
            nc.vector.memset(Rp2[0:64, :, 0:50], 0)
            nc.vector.memset(Rp2[0:64, :, 210:212], 0)
            for t in Cring + Uring:
                nc.vector.memset(t[:, :, 0:1], 0)
                nc.vector.memset(t[:, :, 161:162], 0)
            nc.vector.memset(Uring[0][0:64, :, :].bitcast(I32), 0)

            # ---- phase 1: down() ----
            def phase1(side):
                tgt, c0 = (Lp2, 1) if side == 0 else (Rp2, 50)
                for c5 in range(5):
                    r = 2 * c5
                    ps = psd_p.tile([128, 2, W], F32, tag="psd")
                    for k in range(8):
                        nc.tensor.matmul(
                            ps[:],
                            w1t[:, k, :],
                            xt[side][:, k, r : r + 2, :],
                            start=(k == 0),
                            stop=False,
                        )
                    # bias masked per-row: psum += b1 (x) rowmask -> invalid
                    # rows stay exactly zero through relu
                    nc.tensor.matmul(
                        ps[:],
                        b1r[:],
                        rmw[:, r : r + 2, :],
                        start=False,
                        stop=True,
                    )
                    nc.scalar.activation(
                        tgt[0:64, r : r + 2, c0 : c0 + W], ps[0:64], RELU
                    )
                    if c5 > 0:
                        # bottom half holds rows shifted by 2 (kh2 K-packing)
                        nc.scalar.activation(
                            tgt[64:128, r - 2 : r, c0 : c0 + W], ps[64:128], RELU
                        )

            phase1(0)

            # ---- phase 2 ----
            # G_L: per 2-row chunk, var0 solo then (var1,var2) packed
            for r in (0, 2, 4, 6):
                ps = psg_p.tile([128, 2, W], F32, tag="psg")
                for kw in range(3):
                    nc.tensor.matmul(
                        ps[:],
                        wgl0[:, kw, :],
                        Lp2[:, r : r + 2, kw : kw + W],
                        start=(kw == 0),
                        stop=False,
                    )
                for kw in range(3):
                    nc.tensor.matmul(
                        ps[:],
                        wgl0b[:, kw, :],
                        Lp2[0:64, r + 1 : r + 3, kw : kw + W],
                        start=False,
                        stop=(kw == 2),
                    )
                nc.scalar.activation(Gld[:, r : r + 2, :], ps[:], IDENT, bias=b3a2[:])
                ps2 = psg_p.tile([128, 2, W], F32, tag="psg")
                for kw in range(3):
                    nc.tensor.matmul(
                        ps2[:],
                        wgl12[:, kw, :],
                        Lp2[:, r : r + 2, kw : kw + W],
                        start=(kw == 0),
                        stop=False,
                    )
                for kw in range(3):
                    nc.tensor.matmul(
                        ps2[:],
                        wgl12b[:, kw, :],
                        Lp2[0:64, r + 1 : r + 3, kw : kw + W],
                        start=False,
                        stop=(kw == 2),
                    )
                nc.scalar.activation(Gle[:, r : r + 2, :], ps2[:], IDENT, bias=b3a2[:])

            # F groups
            for g, (fa, fb) in enumerate(FGROUPS):
                qs = [q for q, (gg, _) in enumerate(FQ) if gg == g]
                ps = psg_p.tile([128, ROWS_A, FW], F32, tag="psg")
                n = 2 * len(qs)
                i = 0
                for q in qs:
                    kw = FQ[q][1]
                    nc.tensor.matmul(
                        ps[:],
                        wf[:, q, :],
                        Lp2[:, 0:ROWS_A, kw : kw + FW],
                        start=(i == 0),
                        stop=False,
                    )
                    i += 1
                for q in qs:
                    kw = FQ[q][1]
                    nc.tensor.matmul(
                        ps[:],
                        wfb[:, q, :],
                        Lp2[0:64, 1 : 1 + ROWS_A, kw : kw + FW],
                        start=False,
                        stop=(i == n - 1),
                    )
                    i += 1
                if g == 0:
                    nc.scalar.activation(Fts[0][0:64], ps[0:64], IDENT)
                    nc.scalar.activation(Fts[1][64:128], ps[64:128], IDENT)
                elif g == 1:
                    nc.scalar.activation(Fts[2][0:64], ps[0:64], IDENT)
                    nc.scalar.activation(FE[1][64:128], ps[64:128], IDENT)
                elif g == 2:
                    nc.scalar.activation(FE[0][:], ps[:], IDENT)
                elif g == 3:
                    nc.scalar.activation(FE[1][0:64], ps[0:64], IDENT)
                    nc.scalar.activation(Fts[3][64:128], ps[64:128], IDENT)
                else:
                    nc.scalar.activation(FE[2][64:128], ps[64:128], IDENT)
            nc.scalar.dma_start(Fts[0][64:128, :, :], Fts[0][0:64, :, :])
            nc.scalar.dma_start(Fts[1][0:64, :, :], Fts[1][64:128, :, :])
            nc.scalar.dma_start(Fts[2][64:128, :, :], Fts[2][0:64, :, :])
            nc.scalar.dma_start(Fts[3][0:64, :, :], Fts[3][64:128, :, :])

            phase1(1)

            # E
            for r in (0, 2, 4, 6):
                ps = psg_p.tile([128, 2, GW], F32, tag="psg")
                for si, s in enumerate(S5):
                    nc.tensor.matmul(
                        ps[:],
                        wgr0[:, si, :],
                        Rp2[:, r : r + 2, 40 + s : 40 + s + GW],
                        start=(si == 0),
                        stop=False,
                    )
                for si, s in enumerate(S5):
                    nc.tensor.matmul(
                        ps[:],
                        wgr0b[:, si, :],
                        Rp2[0:64, r + 1 : r + 3, 40 + s : 40 + s + GW],
                        start=False,
                        stop=(si == 4),
                    )
                nc.vector.tensor_add(
                    Grd[:, r : r + 2, :], ps[:], _bcast0(grm[:, r : r + 2], GW)
                )
                ps2 = psg_p.tile([128, 2, GW], F32, tag="psg")
                for si, s in enumerate(S5):
                    nc.tensor.matmul(
                        ps2[:],
                        wgr12[:, si, :],
                        Rp2[:, r : r + 2, 40 + s : 40 + s + GW],
                        start=(si == 0),
                        stop=False,
                    )
                for si, s in enumerate(S5):
                    nc.tensor.matmul(
                        ps2[:],
                        wgr12b[:, si, :],
                        Rp2[0:64, r + 1 : r + 3, 40 + s : 40 + s + GW],
                        start=False,
                        stop=(si == 4),
                    )
                nc.vector.tensor_add(
                    Gre[:, r : r + 2, :], ps2[:], _bcast0(grm[:, r : r + 2], GW)
                )

            # E
            psE = psg_p.tile([128, ROWS_A, EW], F32, tag="psg")
            for kd in range(3):
                nc.tensor.matmul(
                    psE[:],
                    we0[:, kd, :],
                    Rp2[:, 0:ROWS_A, 164 - kd : 164 - kd + EW],
                    start=(kd == 0),
                    stop=False,
                )
            for kd in range(3):
                nc.tensor.matmul(
                    psE[:],
                    we0b[:, kd, :],
                    Rp2[0:64, 1 : 1 + ROWS_A, 164 - kd : 164 - kd + EW],
                    start=False,
                    stop=(kd == 2),
                )
            nc.scalar.activation(Etd[:], psE[:], IDENT)
            psE2 = psg_p.tile([128, ROWS_A, EW], F32, tag="psg")
            for kd in range(3):
                nc.tensor.matmul(
                    psE2[:],
                    we12[:, kd, :],
                    Rp2[:, 0:ROWS_A, 164 - kd : 164 - kd + EW],
                    start=(kd == 0),
                    stop=False,
                )
            for kd in range(3):
                nc.tensor.matmul(
                    psE2[:],
                    we12b[:, kd, :],
                    Rp2[0:64, 1 : 1 + ROWS_A, 164 - kd : 164 - kd + EW],
                    start=False,
                    stop=(kd == 2),
                )
            nc.scalar.activation(Ete[:], psE2[:], IDENT)

            # band-sub tile lookup (var, u) -> tile (slice by C half)
            BANDT = {
                (0, -2): Fts[0], (0, -1): Fts[1], (0, 0): Fts[2], (0, 1): Fts[3],
                (1, 0): FE[0], (1, 1): FE[1],
                (2, -2): FE[0], (2, -1): FE[1], (2, 0): FE[2],
            }

            # ---- d-loop ----
            # C-matmuls of pair p run at iter p+1, U-matmuls + evac at p+2:
            # every PE input is produced at least one iteration earlier.
            ps_open = {}
            for it in range(26):
                if it == 24:
                    nc.vector.memset(Uring[23 % NRING][64:128, :, :].bitcast(I32), 0)
                if it >= 2:
                    p = it - 2
                    d0 = 2 * p
                    wlo = max(0, d0 - 4)
                    width = W - wlo
                    Up = Uring[p % NRING]
                    for j0 in (0, 3):
                        ps = ps_open.pop((p, j0))
                        for t in range(9):
                            kh, kw = divmod(t, 3)
                            nc.tensor.matmul(
                                ps[:, :, 0:width],
                                wut[:, t, :],
                                Up[:, j0 + kh : j0 + kh + 3, wlo + kw : wlo + kw + width],
                                start=False,
                                stop=(t == 8),
                            )
                        ost = ostp.tile([128, 3, W], F32, tag="ost")
                        nc.scalar.activation(
                            ost[:, :, 0:width], ps[:, :, 0:width], RELU, bias=b3b2[:]
                        )
                        nc.sync.dma_start(
                            y_d[d0 : d0 + 2, :, j0 : j0 + 3, wlo:W],
                            ost[:, :, 0:width],
                        )
                if it < 24:
                    j = it
                    Cj = Cring[j % NRING]
                    for d in (2 * j, 2 * j + 1):
                        half = d % 2
                        sl = slice(0, 64) if half == 0 else slice(64, 128)
                        var = _var(d)
                        Glt = Gld if var == 0 else Gle
                        Grt = Grd if var == 0 else Gre
                        Ett = Etd if var == 0 else Ete
                        alo = max(0, d - 10)
                        blo = max(0, d - 2)
                        # region1 first: keeps the ACT queue ahead of DVE
                        if blo > alo:
                            nc.scalar.activation(
                                Cj[sl, :, 1 + alo : 1 + blo],
                                Grt[sl, :, alo - d - GVLO : blo - d - GVLO],
                                RELU,
                                bias=b3a2[sl],
                            )
                        # region2: [blo, W)
                        nc.vector.tensor_add(
                            Cj[sl, :, 1 + blo : 161],
                            Glt[sl, :, blo:W],
                            Grt[sl, :, blo - d - GVLO : W - d - GVLO],
                        )
                        for u in (-2, -1, 0, 1):
                            w = d + u
                            ft = BANDT.get((var, u))
                            if ft is not None and 0 <= w < W:
                                nc.vector.tensor_sub(
                                    Cj[sl, :, 1 + w : 2 + w],
                                    Cj[sl, :, 1 + w : 2 + w],
                                    ft[sl, :, w : w + 1],
                                )
                        nc.vector.tensor_sub(
                            Cj[sl, :, 160:161],
                            Cj[sl, :, 160:161],
                            Ett[sl, :, 47 - d : 48 - d],
                        )
                        nc.vector.tensor_scalar_max(
                            Cj[sl, :, 1 + blo : 161], Cj[sl, :, 1 + blo : 161], 0.0
                        )
                    if j > 0:
                        nc.sync.dma_start(
                            Uring[j % NRING][0:64, :, :],
                            Cring[(j - 1) % NRING][64:128, :, :],
                        )
                        nc.sync.dma_start(
                            Uring[(j - 1) % NRING][64:128, :, :], Cj[0:64, :, :]
                        )
                if 1 <= it <= 24:
                    p = it - 1
                    wlo = max(0, 2 * p - 4)
                    width = W - wlo
                    Cp = Cring[p % NRING]
                    for j0 in (0, 3):
                        ps = ps3_p.tile([128, 3, W], F32, tag="ps3")
                        ps_open[(p, j0)] = ps
                        for t in range(9):
                            kh, kw = divmod(t, 3)
                            nc.tensor.matmul(
                                ps[:, :, 0:width],
                                wct[:, t, :],
                                Cp[:, j0 + kh : j0 + kh + 3, wlo + kw : wlo + kw + width],
                                start=(t == 0),
                                stop=False,
                            )

    nc.finalize()
    return nc


_NC_CACHE = None


def _get_nc():
    global _NC_CACHE
    if _NC_CACHE is None:
        _NC_CACHE = build_nc()
    return _NC_CACHE


def _prep_weights(inputs):
    w1, b1 = _fold_bn(
        inputs["conv1_w"], inputs["conv1_b"], inputs["bn1_g"], inputs["bn1_b"],
        inputs["bn1_m"], inputs["bn1_v"],
    )
    w3a, b3a = _fold_bn(
        inputs["c3a_w"], inputs["c3a_b"], inputs["bn3a_g"], inputs["bn3a_b"],
        inputs["bn3a_m"], inputs["bn3a_v"],
    )
    w3b, b3b = _fold_bn(
        inputs["c3b_w"], inputs["c3b_b"], inputs["bn3b_g"], inputs["bn3b_b"],
        inputs["bn3b_m"], inputs["bn3b_v"],
    )
    wl, wr = w3a[:, :CF], w3a[:, CF:]
    wlq = _bf(wl).astype(np.float32)
    wrq = _bf(wr).astype(np.float32)
    w3bq = _bf(w3b).astype(np.float32)

    def mdup(a):
        """Duplicate M columns: [..., 64] -> [..., 128] with both halves equal."""
        return np.concatenate([a, a], axis=-1)

    out = {}
    out["w1t"] = _bf(
        mdup(np.ascontiguousarray(w1.T.reshape(8, 128, CF).transpose(1, 0, 2)))
    )

    # G_L
    kv = {
        v: sum(wlq[:, :, kd] for kd in kds) for v, kds in KDSETS.items()
    }  # [o,i,kh,kw]
    wgl0 = np.zeros((128, 3, CF), np.float32)
    wgl0b = np.zeros((64, 3, CF), np.float32)
    # duplicated to [*, 3, 128] after fill (see below)
    wgl12 = np.zeros((128, 3, 128), np.float32)
    wgl12b = np.zeros((64, 3, 128), np.float32)
    for kw in range(3):
        wgl0[0:64, kw, :] = kv[0][:, :, 0, kw].T
        wgl0[64:128, kw, :] = kv[0][:, :, 2, kw].T
        wgl0b[:, kw, :] = kv[0][:, :, 1, kw].T
        wgl12[0:64, kw, 0:64] = kv[1][:, :, 0, kw].T
        wgl12[64:128, kw, 0:64] = kv[1][:, :, 2, kw].T
        wgl12[0:64, kw, 64:128] = kv[2][:, :, 0, kw].T
        wgl12[64:128, kw, 64:128] = kv[2][:, :, 2, kw].T
        wgl12b[:, kw, 0:64] = kv[1][:, :, 1, kw].T
        wgl12b[:, kw, 64:128] = kv[2][:, :, 1, kw].T
    out["wgl0"], out["wgl0b"] = _bf(mdup(wgl0)), _bf(mdup(wgl0b))
    out["wgl12"], out["wgl12b"] = _bf(wgl12), _bf(wgl12b)

    # G_R
    def gv(v, s, kh):
        acc = np.zeros((CF, CF), np.float32)
        for kd in KDSETS[v]:
            kw = s + kd
            if 0 <= kw < 3:
                acc += wrq[:, :, kd, kh, kw]
        return acc.T  # [i, o]

    wgr0 = np.zeros((128, 5, CF), np.float32)
    wgr0b = np.zeros((64, 5, CF), np.float32)
    wgr12 = np.zeros((128, 5, 128), np.float32)
    wgr12b = np.zeros((64, 5, 128), np.float32)
    for si, s in enumerate(S5):
        wgr0[0:64, si, :] = gv(0, s, 0)
        wgr0[64:128, si, :] = gv(0, s, 2)
        wgr0b[:, si, :] = gv(0, s, 1)
        wgr12[0:64, si, 0:64] = gv(1, s, 0)
        wgr12[64:128, si, 0:64] = gv(1, s, 2)
        wgr12[0:64, si, 64:128] = gv(2, s, 0)
        wgr12[64:128, si, 64:128] = gv(2, s, 2)
        wgr12b[:, si, 0:64] = gv(1, s, 1)
        wgr12b[:, si, 64:128] = gv(2, s, 1)
    out["wgr0"], out["wgr0b"] = _bf(mdup(wgr0)), _bf(mdup(wgr0b))
    out["wgr12"], out["wgr12b"] = _bf(wgr12), _bf(wgr12b)

    # E
    def ev(v, kd, kh):
        if kd in KDSETS[v]:
            return wrq[:, :, kd, kh, 2].T
        return np.zeros((CF, CF), np.float32)

    we0 = np.zeros((128, 3, CF), np.float32)
    we0b = np.zeros((64, 3, CF), np.float32)
    we12 = np.zeros((128, 3, 128), np.float32)
    we12b = np.zeros((64, 3, 128), np.float32)
    for kd in range(3):
        we0[0:64, kd, :] = ev(0, kd, 0)
        we0[64:128, kd, :] = ev(0, kd, 2)
        we0b[:, kd, :] = ev(0, kd, 1)
        we12[0:64, kd, 0:64] = ev(1, kd, 0)
        we12[64:128, kd, 0:64] = ev(1, kd, 2)
        we12[0:64, kd, 64:128] = ev(2, kd, 0)
        we12[64:128, kd, 64:128] = ev(2, kd, 2)
        we12b[:, kd, 0:64] = ev(1, kd, 1)
        we12b[:, kd, 64:128] = ev(2, kd, 1)
    out["we0"], out["we0b"] = _bf(mdup(we0)), _bf(mdup(we0b))
    out["we12"], out["we12b"] = _bf(we12), _bf(we12b)

    # F
    def fwf(fi, kh, kw):
        var, u, kws = F_COMBOS[fi]
        acc = np.zeros((CF, CF), np.float32)
        for kd in KDSETS[var]:
            if kd > u + kw:
                acc += wlq[:, :, kd, kh, kw]
        return acc.T

    wfq = np.zeros((128, NFQ, 128), np.float32)
    wfbq = np.zeros((64, NFQ, 128), np.float32)
    for q, (g, kw) in enumerate(FQ):
        fa, fb = FGROUPS[g]
        if fa is not None:
            wfq[0:64, q, 0:64] = fwf(fa, 0, kw)
            wfq[64:128, q, 0:64] = fwf(fa, 2, kw)
            wfbq[:, q, 0:64] = fwf(fa, 1, kw)
        if fb is not None:
            wfq[0:64, q, 64:128] = fwf(fb, 0, kw)
            wfq[64:128, q, 64:128] = fwf(fb, 2, kw)
            wfbq[:, q, 64:128] = fwf(fb, 1, kw)
    out["wf"], out["wfb"] = _bf(wfq), _bf(wfbq)

    # conv3b pair-packed
    wct = np.zeros((128, 9, 128), np.float32)
    wut = np.zeros((128, 9, 128), np.float32)
    for t in range(9):
        kh, kw = divmod(t, 3)
        wct[0:64, t, 0:64] = w3bq[:, :, 1, kh, kw].T
        wct[0:64, t, 64:128] = w3bq[:, :, 0, kh, kw].T
        wct[64:128, t, 0:64] = w3bq[:, :, 2, kh, kw].T
        wct[64:128, t, 64:128] = w3bq[:, :, 1, kh, kw].T
        wut[0:64, t, 0:64] = w3bq[:, :, 0, kh, kw].T
        wut[64:128, t, 64:128] = w3bq[:, :, 2, kh, kw].T
    out["wct"], out["wut"] = _bf(wct), _bf(wut)

    out["b1c"] = np.concatenate([b1, b1]).reshape(128, 1)
    out["b1r"] = _bf(np.concatenate([b1, b1]).reshape(1, 128))
    out["b3a2"] = np.concatenate([b3a, b3a]).reshape(128, 1)
    out["b3b2"] = np.concatenate([b3b, b3b]).reshape(128, 1)
    out["_b3a"] = b3a
    out["_b3b"] = b3b
    out["_w3b"] = w3bq
    return out


def _host_y_init(w3bq, b3a, b3b, c):
    """Const-filled initial y [D, CF, HLOC, W] for core c (f32)."""
    a0 = np.maximum(b3a, 0.0)
    y = np.zeros((D, CF, HLOC, W), np.float32)
    pre = np.einsum("oikhw,i->okhw", w3bq, a0)  # [o, kd, kh, kw]
    for r in range(HLOC):
        g = 6 * c + r
        khs = [kh for kh in range(3) if 0 <= g + kh - 1 < H]
        for d in range(D):
            wlo = max(0, d - 4)
            if wlo == 0:
                continue
            kds = [kd for kd in range(3) if 0 <= d + kd - 1 < D]
            cint = np.maximum(pre[:, kds][:, :, khs].sum((1, 2, 3)) + b3b, 0.0)
            cw0 = np.maximum(
                pre[:, kds][:, :, khs][:, :, :, 1:].sum((1, 2, 3)) + b3b, 0.0
            )
            y[d, :, r, 1:wlo] = cint[:, None]
            y[d, :, r, 0] = cw0
    return y


def _per_core_inputs(inputs, shared, c):
    r0 = 6 * c
    rows = np.arange(r0 - 2, r0 + 8)
    valid = (rows >= 0) & (rows < H)

    def slc(x):
        out = np.zeros((CIN, ROWS_IN, W), np.float32)
        out[:, valid] = x[0][:, rows[valid]]
        return _bf(out.reshape(CIN, ROWS_IN * W))

    m = {k: v for k, v in shared.items() if not k.startswith("_")}
    m["xl"] = slc(np.asarray(inputs["left_features"], np.float32))
    m["xr"] = slc(np.asarray(inputs["right_features"], np.float32))
    m["rowm"] = _bf(np.broadcast_to(valid.astype(np.float32), (128, ROWS_IN)))
    m["rmw"] = _bf(
        np.broadcast_to(valid.astype(np.float32)[None, :, None], (1, ROWS_IN, W))
    )
    arows = np.arange(r0 - 1, r0 + 7)
    gvals = np.where((arows >= 0) & (arows < H), 0.0, NEG).astype(np.float32)
    m["grm"] = np.broadcast_to(gvals, (128, ROWS_A)).copy()
    return m


_EXEC_CACHE = None


def _get_exec():
    """Build the SPMD executable once; reuse across kernel() calls."""
    global _EXEC_CACHE
    if _EXEC_CACHE is not None:
        return _EXEC_CACHE
    import jax
    import concourse.mybir as mb
    from concourse import bass2jax
    from jax.experimental.shard_map import shard_map
    from jax.sharding import Mesh, PartitionSpec

    nc = _get_nc()
    bass2jax.install_neuronx_cc_hook()
    partition_name = nc.partition_id_tensor.name if nc.partition_id_tensor else None
    in_names, out_names, out_avals = [], [], []
    for alloc in nc.m.functions[0].allocations:
        if not isinstance(alloc, mb.MemoryLocationSet):
            continue
        name = alloc.memorylocations[0].name
        if alloc.kind == "ExternalInput":
            if name != partition_name:
                in_names.append(name)
        elif alloc.kind == "ExternalOutput":
            shape = tuple(alloc.tensor_shape)
            dtype = mb.dt.np(alloc.dtype)
            out_names.append(name)
            out_avals.append(jax.core.ShapedArray(shape, dtype))
    n_params = len(in_names)
    all_in = list(in_names) + list(out_names)
    if partition_name is not None:
        all_in.append(partition_name)

    def _body(*args):
        operands = list(args)
        if partition_name is not None:
            operands.append(bass2jax.partition_id_tensor())
        outs = bass2jax._bass_exec_p.bind(
            *operands,
            out_avals=tuple(out_avals),
            in_names=tuple(all_in),
            out_names=tuple(out_names),
            lowering_input_output_aliases=(),
            sim_require_finite=True,
            sim_require_nnan=True,
            nc=nc,
        )
        return tuple(outs)

    devices = jax.devices()[:NC]
    mesh = Mesh(np.asarray(devices), ("core",))
    n_outs = len(out_names)
    sharded = jax.jit(
        shard_map(
            _body,
            mesh=mesh,
            in_specs=(PartitionSpec("core"),) * (n_params + n_outs),
            out_specs=(PartitionSpec("core"),) * n_outs,
            check_rep=False,
        ),
        donate_argnums=tuple(range(n_params, n_params + n_outs)),
        keep_unused=True,
    )
    _EXEC_CACHE = (sharded, in_names, out_names, out_avals)
    return _EXEC_CACHE


def _run(in_maps, out_inits):
    sharded, in_names, out_names, out_avals = _get_exec()
    concat_in = [
        np.concatenate([np.asarray(in_maps[c][nm]) for c in range(NC)], axis=0)
        for nm in in_names
    ]
    concat_outs = [
        np.concatenate([np.asarray(out_inits[c][nm]) for c in range(NC)], axis=0)
        for nm in out_names
    ]
    out_arrs = sharded(*concat_in, *concat_outs)
    return [
        {
            nm: np.asarray(out_arrs[i]).reshape(NC, *out_avals[i].shape)[c]
            for i, nm in enumerate(out_names)
        }
        for c in range(NC)
    ]


def kernel(**inputs):
    shared = _prep_weights(inputs)
    in_maps = [_per_core_inputs(inputs, shared, c) for c in range(NC)]
    out_inits = [
        {"y": _host_y_init(shared["_w3b"], shared["_b3a"], shared["_b3b"], c)}
        for c in range(NC)
    ]
    results = _run(in_maps, out_inits)
    full = np.zeros((CF, D, H, W), np.float32)
    for c in range(NC):
        y = results[c]["y"]  # [48, 64, 6, 160]
        full[:, :, 6 * c : 6 * c + 6, :] = y.transpose(1, 0, 2, 3)
    return full.reshape(1, CF * D, H, W)


# revision 25
# speedup vs baseline: 1.0575x; 1.0575x over previous
"""Trainium2 Bass kernel for nn_CostVolume: H-sharded across 8 NeuronCores.

v2: bf16 matmuls + M=128 pair-packing for conv3b + narrowed compute.

- BN folded into conv weights on host; all matmul operands bf16 (end-to-end
  rel err ~5e-3 vs fp32 reference, tolerance 2e-2).
- down(): 1x1 conv K=1024; x shipped bf16, loaded as 8 big DMAs per side.
- conv3a collapsed into G_L/G_R/E/F 2D convs (cost-volume shift structure);
  K-packed over (kh0,kh2) via row-shifted stacked tiles Lp2/Rp2, M-packed
  over variant pairs. b3a baked into G_L evac.
- A[d] assembled per-d by DVE (add/sub/relu) into paired tiles
  C_j = [A[2j]; A[2j+1]]; U_j = [A[2j-1]; A[2j+2]] built by 2 half-copies.
- conv3b M=128 pair-packed: outputs (2j, 2j+1) computed together, 18 matmuls
  per (pair, 3-row chunk) instead of 30: 9 taps on C_j ([w1;w0],[w2;w1]) and
  9 on U_j ([w0;0],[0;w2]).
- w < d-4 output region is constant (masked cost volume): skipped on-device,
  pre-filled host-side in the initial y buffer; A tiles rely on ring-stale
  columns matching the same constant.
"""

import sys

sys.path.insert(0, "/opt/trn_rl_repo")

import numpy as np
import ml_dtypes
import concourse.bass as bass
import concourse.bacc as bacc
import concourse.mybir as mybir
from concourse import tile

F32 = mybir.dt.float32
BF = mybir.dt.bfloat16
I32 = mybir.dt.int32
RELU = mybir.ActivationFunctionType.Relu
IDENT = mybir.ActivationFunctionType.Identity

H, W, D, CF, CIN = 48, 160, 48, 64, 1024
EPS = 1e-5
NC = 8
HLOC = 6
ROWS_IN = 10
ROWS_A = 8
WP = 162
GVLO = -10  # Gr col range [v=-10, 160)
GW = 170
NRING = 4
EV0 = 112
EW = 48
FW = 52
NEG = -1.0e30
S5 = (-2, -1, 0, 1, 2)

KDSETS = {0: (0, 1, 2), 1: (1, 2), 2: (0, 1)}


def _var(d):
    return 1 if d == 0 else (2 if d == D - 1 else 0)


def _fold_bn(w, b, g, beta, m, v):
    s = (g / np.sqrt(v + EPS)).astype(np.float32)
    return (w * s.reshape(-1, *([1] * (w.ndim - 1)))).astype(np.float32), (
        (b - m) * s + beta
    ).astype(np.float32)


def _f_combos():
    combos = []
    for var, kds in KDSETS.items():
        urange = (0, 1) if var == 1 else (-2, -1, 0, 1)
        for u in urange:
            kws = [kw for kw in range(3) if any(kd > u + kw for kd in kds)]
            if kws:
                combos.append((var, u, kws))
    return combos


F_COMBOS = _f_combos()
FIDX = {(var, u): fi for fi, (var, u, _) in enumerate(F_COMBOS)}
# M-pack groups for F: (top fi, bottom fi), kw list per group
FGROUPS = [(0, 1), (2, 7), (4, 6), (5, 3), (None, 8)]
FKWL = [[0, 1, 2], [0, 1], [0, 1, 2], [0], [0]]
FQ = [(g, kw) for g, kws in enumerate(FKWL) for kw in kws]  # 10 flat taps
NFQ = len(FQ)


def _bcast0(ap, n):
    return bass.AP(ap.tensor, ap.offset, list(ap.ap) + [[0, n]])


def _bf(x):
    return np.asarray(x, np.float32).astype(ml_dtypes.bfloat16)


def build_nc():
    nc = bacc.Bacc("TRN2", target_bir_lowering=False, debug=False, num_devices=NC)

    xl_d = nc.dram_tensor("xl", [CIN, ROWS_IN * W], BF, kind="ExternalInput")
    xr_d = nc.dram_tensor("xr", [CIN, ROWS_IN * W], BF, kind="ExternalInput")
    w1t_d = nc.dram_tensor("w1t", [128, 8, 128], BF, kind="ExternalInput")
    wgl0_d = nc.dram_tensor("wgl0", [128, 3, 128], BF, kind="ExternalInput")
    wgl0b_d = nc.dram_tensor("wgl0b", [64, 3, 128], BF, kind="ExternalInput")
    wgl12_d = nc.dram_tensor("wgl12", [128, 3, 128], BF, kind="ExternalInput")
    wgl12b_d = nc.dram_tensor("wgl12b", [64, 3, 128], BF, kind="ExternalInput")
    wgr0_d = nc.dram_tensor("wgr0", [128, 5, 128], BF, kind="ExternalInput")
    wgr0b_d = nc.dram_tensor("wgr0b", [64, 5, 128], BF, kind="ExternalInput")
    wgr12_d = nc.dram_tensor("wgr12", [128, 5, 128], BF, kind="ExternalInput")
    wgr12b_d = nc.dram_tensor("wgr12b", [64, 5, 128], BF, kind="ExternalInput")
    we0_d = nc.dram_tensor("we0", [128, 3, 128], BF, kind="ExternalInput")
    we0b_d = nc.dram_tensor("we0b", [64, 3, 128], BF, kind="ExternalInput")
    we12_d = nc.dram_tensor("we12", [128, 3, 128], BF, kind="ExternalInput")
    we12b_d = nc.dram_tensor("we12b", [64, 3, 128], BF, kind="ExternalInput")
    wf_d = nc.dram_tensor("wf", [128, NFQ, 128], BF, kind="ExternalInput")
    wfb_d = nc.dram_tensor("wfb", [64, NFQ, 128], BF, kind="ExternalInput")
    wct_d = nc.dram_tensor("wct", [128, 9, 128], BF, kind="ExternalInput")
    wut_d = nc.dram_tensor("wut", [128, 9, 128], BF, kind="ExternalInput")
    b1c_d = nc.dram_tensor("b1c", [128, 1], F32, kind="ExternalInput")
    b1r_d = nc.dram_tensor("b1r", [1, 128], BF, kind="ExternalInput")
    rmw_d = nc.dram_tensor("rmw", [1, ROWS_IN, W], BF, kind="ExternalInput")
    b3a2_d = nc.dram_tensor("b3a2", [128, 1], F32, kind="ExternalInput")
    b3b2_d = nc.dram_tensor("b3b2", [128, 1], F32, kind="ExternalInput")
    rowm_d = nc.dram_tensor("rowm", [128, ROWS_IN], BF, kind="ExternalInput")
    grm_d = nc.dram_tensor("grm", [128, ROWS_A], F32, kind="ExternalInput")
    y_d = nc.dram_tensor("y", [D, CF, HLOC, W], F32, kind="ExternalOutput")

    with tile.TileContext(nc) as tc:
        with (
            tc.tile_pool(name="wpool", bufs=1) as wpool,
            tc.tile_pool(name="big", bufs=1) as big,
            tc.tile_pool(name="ost", bufs=4) as ostp,
            tc.tile_pool(name="psd", bufs=2, space="PSUM") as psd_p,
            tc.tile_pool(name="psg", bufs=2, space="PSUM") as psg_p,
            tc.tile_pool(name="ps3", bufs=4, space="PSUM") as ps3_p,
        ):
            # ---- persistent tiles ----
            xt = [big.tile([128, 8, ROWS_IN, W], BF, name=f"xt{s}") for s in range(2)]

            # ---- input/weight DMAs, ordered for startup overlap ----
            def xload(side, x_d, r0, r1):
                # one DMA covering all 8 K-chunks for rows [r0, r1)
                nc.sync.dma_start(
                    xt[side][:, :, r0:r1, :],
                    x_d[:, W * r0 : W * r1].rearrange(
                        "(k p) (r c) -> p k r c", p=128, r=r1 - r0
                    ),
                )

            def wt(dram, shape):
                t = wpool.tile(shape, BF, name=dram.name + "_t")
                nc.sync.dma_start(t[:], dram[:])
                return t

            w1t = wt(w1t_d, [128, 8, 128])
            b1r = wt(b1r_d, [1, 128])
            rmw = wt(rmw_d, [1, ROWS_IN, W])
            xload(0, xl_d, 0, 2)
            for r0 in (2, 4, 6, 8):
                xload(0, xl_d, r0, r0 + 2)
            for r0 in (0, 2, 4, 6, 8):
                xload(1, xr_d, r0, r0 + 2)
            wgl0 = wt(wgl0_d, [128, 3, 128])
            wgl0b = wt(wgl0b_d, [64, 3, 128])
            wgl12 = wt(wgl12_d, [128, 3, 128])
            wgl12b = wt(wgl12b_d, [64, 3, 128])
            rowm = wpool.tile([128, ROWS_IN], BF)
            nc.sync.dma_start(rowm[:], rowm_d[:])
            b3a2 = wpool.tile([128, 1], F32)
            nc.sync.dma_start(b3a2[:], b3a2_d[:])
            wf = wt(wf_d, [128, NFQ, 128])
            wfb = wt(wfb_d, [64, NFQ, 128])
            wgr0 = wt(wgr0_d, [128, 5, 128])
            wgr0b = wt(wgr0b_d, [64, 5, 128])
            wgr12 = wt(wgr12_d, [128, 5, 128])
            wgr12b = wt(wgr12b_d, [64, 5, 128])
            grm = wpool.tile([128, ROWS_A], F32)
            nc.sync.dma_start(grm[:], grm_d[:])
            we0 = wt(we0_d, [128, 3, 128])
            we0b = wt(we0b_d, [64, 3, 128])
            we12 = wt(we12_d, [128, 3, 128])
            we12b = wt(we12b_d, [64, 3, 128])
            wct = wt(wct_d, [128, 9, 128])
            wut = wt(wut_d, [128, 9, 128])
            b3b2 = wpool.tile([128, 1], F32)
            nc.sync.dma_start(b3b2[:], b3b2_d[:])
            Lp2 = big.tile([128, ROWS_IN, WP], BF)
            Rp2 = big.tile([128, ROWS_IN, 212], BF)
            Gld = big.tile([128, ROWS_A, W], BF)  # var0 both halves
            Gle = big.tile([128, ROWS_A, W], BF)  # top var1, bottom var2
            Grd = big.tile([128, ROWS_A, GW], BF)
            Gre = big.tile([128, ROWS_A, GW], BF)
            Etd = big.tile([128, ROWS_A, EW], BF)
            Ete = big.tile([128, ROWS_A, EW], BF)
            Fts = [big.tile([128, ROWS_A, FW], BF, name=f"Ft{i}") for i in range(4)]
            FE = [big.tile([128, ROWS_A, FW], BF, name=f"FE{i}") for i in range(3)]
            Cring = [
                big.tile([128, ROWS_A, WP], BF, name=f"C{i}") for i in range(NRING)
            ]
            Uring = [
                big.tile([128, ROWS_A, WP], BF, name=f"U{i}") for i in range(NRING)
            ]

            # warm the ACT function table while DMAs stream
            scr = wpool.tile([1, 2], F32, name="scr")
            nc.vector.memset(scr[:], 0)
            nc.scalar.activation(scr[:], scr[:], RELU)

            # pad memsets (before writes)
            nc.vector.memset(Lp2[0:64, :, 0:1], 0)
            nc.vector.memset(Lp2[0:64, :, 161:162], 0)
            nc.vector.memset(Rp2[0:64, :, 0:50], 0)
            nc.vector.memset(Rp2[0:64, :, 210:212], 0)
            for t in Cring + Uring:
                nc.vector.memset(t[:, :, 0:1], 0)
                nc.vector.memset(t[:, :, 161:162], 0)
            nc.vector.memset(Uring[0][0:64, :, :].bitcast(I32), 0)

            # ---- phase 1: down() ----
            def phase1(side):
                tgt, c0 = (Lp2, 1) if side == 0 else (Rp2, 50)
                for c5 in range(5):
                    r = 2 * c5
                    ps = psd_p.tile([128, 2, W], F32, tag="psd")
                    for k in range(8):
                        nc.tensor.matmul(
                            ps[:],
                            w1t[:, k, :],
                            xt[side][:, k, r : r + 2, :],
                            start=(k == 0),
                            stop=False,
                        )
                    # bias masked per-row: psum += b1 (x) rowmask -> invalid
                    # rows stay exactly zero through relu
                    nc.tensor.matmul(
                        ps[:],
                        b1r[:],
                        rmw[:, r : r + 2, :],
                        start=False,
                        stop=True,
                    )
                    nc.scalar.activation(
                        tgt[0:64, r : r + 2, c0 : c0 + W], ps[0:64], RELU
                    )
                    if c5 > 0:
                        # bottom half holds rows shifted by 2 (kh2 K-packing)
                        nc.scalar.activation(
                            tgt[64:128, r - 2 : r, c0 : c0 + W], ps[64:128], RELU
                        )

            phase1(0)

            # ---- phase 2 ----
            # G_L: per 2-row chunk, var0 solo then (var1,var2) packed
            for r in (0, 2, 4, 6):
                ps = psg_p.tile([128, 2, W], F32, tag="psg")
                for kw in range(3):
                    nc.tensor.matmul(
                        ps[:],
                        wgl0[:, kw, :],
                        Lp2[:, r : r + 2, kw : kw + W],
                        start=(kw == 0),
                        stop=False,
                    )
                for kw in range(3):
                    nc.tensor.matmul(
                        ps[:],
                        wgl0b[:, kw, :],
                        Lp2[0:64, r + 1 : r + 3, kw : kw + W],
                        start=False,
                        stop=(kw == 2),
                    )
                nc.scalar.activation(Gld[:, r : r + 2, :], ps[:], IDENT, bias=b3a2[:])
                ps2 = psg_p.tile([128, 2, W], F32, tag="psg")
                for kw in range(3):
                    nc.tensor.matmul(
                        ps2[:],
                        wgl12[:, kw, :],
                        Lp2[:, r : r + 2, kw : kw + W],
                        start=(kw == 0),
                        stop=False,
                    )
                for kw in range(3):
                    nc.tensor.matmul(
                        ps2[:],
                        wgl12b[:, kw, :],
                        Lp2[0:64, r + 1 : r + 3, kw : kw + W],
                        start=False,
                        stop=(kw == 2),
                    )
                nc.scalar.activation(Gle[:, r : r + 2, :], ps2[:], IDENT, bias=b3a2[:])

            # F groups
            for g, (fa, fb) in enumerate(FGROUPS):
                qs = [q for q, (gg, _) in enumerate(FQ) if gg == g]
                ps = psg_p.tile([128, ROWS_A, FW], F32, tag="psg")
                n = 2 * len(qs)
                i = 0
                for q in qs:
                    kw = FQ[q][1]
                    nc.tensor.matmul(
                        ps[:],
                        wf[:, q, :],
                        Lp2[:, 0:ROWS_A, kw : kw + FW],
                        start=(i == 0),
                        stop=False,
                    )
                    i += 1
                for q in qs:
                    kw = FQ[q][1]
                    nc.tensor.matmul(
                        ps[:],
                        wfb[:, q, :],
                        Lp2[0:64, 1 : 1 + ROWS_A, kw : kw + FW],
                        start=False,
                        stop=(i == n - 1),
                    )
                    i += 1
                if g == 0:
                    nc.scalar.activation(Fts[0][0:64], ps[0:64], IDENT)
                    nc.scalar.activation(Fts[1][64:128], ps[64:128], IDENT)
                elif g == 1:
                    nc.scalar.activation(Fts[2][0:64], ps[0:64], IDENT)
                    nc.scalar.activation(FE[1][64:128], ps[64:128], IDENT)
                elif g == 2:
                    nc.scalar.activation(FE[0][:], ps[:], IDENT)
                elif g == 3:
                    nc.scalar.activation(FE[1][0:64], ps[0:64], IDENT)
                    nc.scalar.activation(Fts[3][64:128], ps[64:128], IDENT)
                else:
                    nc.scalar.activation(FE[2][64:128], ps[64:128], IDENT)
            nc.scalar.dma_start(Fts[0][64:128, :, :], Fts[0][0:64, :, :])
            nc.scalar.dma_start(Fts[1][0:64, :, :], Fts[1][64:128, :, :])
            nc.scalar.dma_start(Fts[2][64:128, :, :], Fts[2][0:64, :, :])
            nc.scalar.dma_start(Fts[3][0:64, :, :], Fts[3][64:128, :, :])

            phase1(1)

            # E
            for r in (0, 2, 4, 6):
                ps = psg_p.tile([128, 2, GW], F32, tag="psg")
                for si, s in enumerate(S5):
                    nc.tensor.matmul(
                        ps[:],
                        wgr0[:, si, :],
                        Rp2[:, r : r + 2, 40 + s : 40 + s + GW],
                        start=(si == 0),
                        stop=False,
                    )
                for si, s in enumerate(S5):
                    nc.tensor.matmul(
                        ps[:],
                        wgr0b[:, si, :],
                        Rp2[0:64, r + 1 : r + 3, 40 + s : 40 + s + GW],
                        start=False,
                        stop=(si == 4),
                    )
                nc.vector.tensor_add(
                    Grd[:, r : r + 2, :], ps[:], _bcast0(grm[:, r : r + 2], GW)
                )
                ps2 = psg_p.tile([128, 2, GW], F32, tag="psg")
                for si, s in enumerate(S5):
                    nc.tensor.matmul(
                        ps2[:],
                        wgr12[:, si, :],
                        Rp2[:, r : r + 2, 40 + s : 40 + s + GW],
                        start=(si == 0),
                        stop=False,
                    )
                for si, s in enumerate(S5):
                    nc.tensor.matmul(
                        ps2[:],
                        wgr12b[:, si, :],
                        Rp2[0:64, r + 1 : r + 3, 40 + s : 40 + s + GW],
                        start=False,
                        stop=(si == 4),
                    )
                nc.vector.tensor_add(
                    Gre[:, r : r + 2, :], ps2[:], _bcast0(grm[:, r : r + 2], GW)
                )

            # E
            psE = psg_p.tile([128, ROWS_A, EW], F32, tag="psg")
            for kd in range(3):
                nc.tensor.matmul(
                    psE[:],
                    we0[:, kd, :],
                    Rp2[:, 0:ROWS_A, 164 - kd : 164 - kd + EW],
                    start=(kd == 0),
                    stop=False,
                )
            for kd in range(3):
                nc.tensor.matmul(
                    psE[:],
                    we0b[:, kd, :],
                    Rp2[0:64, 1 : 1 + ROWS_A, 164 - kd : 164 - kd + EW],
                    start=False,
                    stop=(kd == 2),
                )
            nc.scalar.activation(Etd[:], psE[:], IDENT)
            psE2 = psg_p.tile([128, ROWS_A, EW], F32, tag="psg")
            for kd in range(3):
                nc.tensor.matmul(
                    psE2[:],
                    we12[:, kd, :],
                    Rp2[:, 0:ROWS_A, 164 - kd : 164 - kd + EW],
                    start=(kd == 0),
                    stop=False,
                )
            for kd in range(3):
                nc.tensor.matmul(
                    psE2[:],
                    we12b[:, kd, :],
                    Rp2[0:64, 1 : 1 + ROWS_A, 164 - kd : 164 - kd + EW],
                    start=False,
                    stop=(kd == 2),
                )
            nc.scalar.activation(Ete[:], psE2[:], IDENT)

            # band-sub tile lookup (var, u) -> tile (slice by C half)
            BANDT = {
                (0, -2): Fts[0], (0, -1): Fts[1], (0, 0): Fts[2], (0, 1): Fts[3],
                (1, 0): FE[0], (1, 1): FE[1],
                (2, -2): FE[0], (2, -1): FE[1], (2, 0): FE[2],
            }

            # ---- d-loop ----
            # C-matmuls of pair p run at iter p+1, U-matmuls + evac at p+2:
            # every PE input is produced at least one iteration earlier.
            ps_open = {}
            for it in range(26):
                if it == 24:
                    nc.vector.memset(Uring[23 % NRING][64:128, :, :].bitcast(I32), 0)
                if it < 24:
                    j = it
                    Cj = Cring[j % NRING]
                    for d in (2 * j, 2 * j + 1):
                        half = d % 2
                        sl = slice(0, 64) if half == 0 else slice(64, 128)
                        var = _var(d)
                        Glt = Gld if var == 0 else Gle
                        Grt = Grd if var == 0 else Gre
                        Ett = Etd if var == 0 else Ete
                        alo = max(0, d - 10)
                        blo = max(0, d - 2)
                        # region1 first: keeps the ACT queue ahead of DVE
                        if blo > alo:
                            nc.scalar.activation(
                                Cj[sl, :, 1 + alo : 1 + blo],
                                Grt[sl, :, alo - d - GVLO : blo - d - GVLO],
                                RELU,
                                bias=b3a2[sl],
                            )
                        # region2: [blo, W)
                        nc.vector.tensor_add(
                            Cj[sl, :, 1 + blo : 161],
                            Glt[sl, :, blo:W],
                            Grt[sl, :, blo - d - GVLO : W - d - GVLO],
                        )
                        for u in (-2, -1, 0, 1):
                            w = d + u
                            ft = BANDT.get((var, u))
                            if ft is not None and 0 <= w < W:
                                nc.vector.tensor_sub(
                                    Cj[sl, :, 1 + w : 2 + w],
                                    Cj[sl, :, 1 + w : 2 + w],
                                    ft[sl, :, w : w + 1],
                                )
                        nc.vector.tensor_sub(
                            Cj[sl, :, 160:161],
                            Cj[sl, :, 160:161],
                            Ett[sl, :, 47 - d : 48 - d],
                        )
                        nc.vector.tensor_scalar_max(
                            Cj[sl, :, 1 + blo : 161], Cj[sl, :, 1 + blo : 161], 0.0
                        )
                    if j > 0:
                        nc.sync.dma_start(
                            Uring[j % NRING][0:64, :, :],
                            Cring[(j - 1) % NRING][64:128, :, :],
                        )
                        nc.sync.dma_start(
                            Uring[(j - 1) % NRING][64:128, :, :], Cj[0:64, :, :]
                        )
                if it >= 2:
                    p = it - 2
                    d0 = 2 * p
                    wlo = max(0, d0 - 4)
                    width = W - wlo
                    Up = Uring[p % NRING]
                    for j0 in (0, 3):
                        ps = ps_open.pop((p, j0))
                        for t in range(9):
                            kh, kw = divmod(t, 3)
                            nc.tensor.matmul(
                                ps[:, :, 0:width],
                                wut[:, t, :],
                                Up[:, j0 + kh : j0 + kh + 3, wlo + kw : wlo + kw + width],
                                start=False,
                                stop=(t == 8),
                            )
                        ost = ostp.tile([128, 3, W], F32, tag="ost")
                        nc.scalar.activation(
                            ost[:, :, 0:width], ps[:, :, 0:width], RELU, bias=b3b2[:]
                        )
                        nc.sync.dma_start(
                            y_d[d0 : d0 + 2, :, j0 : j0 + 3, wlo:W],
                            ost[:, :, 0:width],
                        )
                if 1 <= it <= 24:
                    p = it - 1
                    wlo = max(0, 2 * p - 4)
                    width = W - wlo
                    Cp = Cring[p % NRING]
                    for j0 in (0, 3):
                        ps = ps3_p.tile([128, 3, W], F32, tag="ps3")
                        ps_open[(p, j0)] = ps
                        for t in range(9):
                            kh, kw = divmod(t, 3)
                            nc.tensor.matmul(
                                ps[:, :, 0:width],
                                wct[:, t, :],
                                Cp[:, j0 + kh : j0 + kh + 3, wlo + kw : wlo + kw + width],
                                start=(t == 0),
                                stop=False,
                            )

    nc.finalize()
    return nc


_NC_CACHE = None


def _get_nc():
    global _NC_CACHE
    if _NC_CACHE is None:
        _NC_CACHE = build_nc()
    return _NC_CACHE


def _prep_weights(inputs):
    w1, b1 = _fold_bn(
        inputs["conv1_w"], inputs["conv1_b"], inputs["bn1_g"], inputs["bn1_b"],
        inputs["bn1_m"], inputs["bn1_v"],
    )
    w3a, b3a = _fold_bn(
        inputs["c3a_w"], inputs["c3a_b"], inputs["bn3a_g"], inputs["bn3a_b"],
        inputs["bn3a_m"], inputs["bn3a_v"],
    )
    w3b, b3b = _fold_bn(
        inputs["c3b_w"], inputs["c3b_b"], inputs["bn3b_g"], inputs["bn3b_b"],
        inputs["bn3b_m"], inputs["bn3b_v"],
    )
    wl, wr = w3a[:, :CF], w3a[:, CF:]
    wlq = _bf(wl).astype(np.float32)
    wrq = _bf(wr).astype(np.float32)
    w3bq = _bf(w3b).astype(np.float32)

    def mdup(a):
        """Duplicate M columns: [..., 64] -> [..., 128] with both halves equal."""
        return np.concatenate([a, a], axis=-1)

    out = {}
    out["w1t"] = _bf(
        mdup(np.ascontiguousarray(w1.T.reshape(8, 128, CF).transpose(1, 0, 2)))
    )

    # G_L
    kv = {
        v: sum(wlq[:, :, kd] for kd in kds) for v, kds in KDSETS.items()
    }  # [o,i,kh,kw]
    wgl0 = np.zeros((128, 3, CF), np.float32)
    wgl0b = np.zeros((64, 3, CF), np.float32)
    # duplicated to [*, 3, 128] after fill (see below)
    wgl12 = np.zeros((128, 3, 128), np.float32)
    wgl12b = np.zeros((64, 3, 128), np.float32)
    for kw in range(3):
        wgl0[0:64, kw, :] = kv[0][:, :, 0, kw].T
        wgl0[64:128, kw, :] = kv[0][:, :, 2, kw].T
        wgl0b[:, kw, :] = kv[0][:, :, 1, kw].T
        wgl12[0:64, kw, 0:64] = kv[1][:, :, 0, kw].T
        wgl12[64:128, kw, 0:64] = kv[1][:, :, 2, kw].T
        wgl12[0:64, kw, 64:128] = kv[2][:, :, 0, kw].T
        wgl12[64:128, kw, 64:128] = kv[2][:, :, 2, kw].T
        wgl12b[:, kw, 0:64] = kv[1][:, :, 1, kw].T
        wgl12b[:, kw, 64:128] = kv[2][:, :, 1, kw].T
    out["wgl0"], out["wgl0b"] = _bf(mdup(wgl0)), _bf(mdup(wgl0b))
    out["wgl12"], out["wgl12b"] = _bf(wgl12), _bf(wgl12b)

    # G_R
    def gv(v, s, kh):
        acc = np.zeros((CF, CF), np.float32)
        for kd in KDSETS[v]:
            kw = s + kd
            if 0 <= kw < 3:
                acc += wrq[:, :, kd, kh, kw]
        return acc.T  # [i, o]

    wgr0 = np.zeros((128, 5, CF), np.float32)
    wgr0b = np.zeros((64, 5, CF), np.float32)
    wgr12 = np.zeros((128, 5, 128), np.float32)
    wgr12b = np.zeros((64, 5, 128), np.float32)
    for si, s in enumerate(S5):
        wgr0[0:64, si, :] = gv(0, s, 0)
        wgr0[64:128, si, :] = gv(0, s, 2)
        wgr0b[:, si, :] = gv(0, s, 1)
        wgr12[0:64, si, 0:64] = gv(1, s, 0)
        wgr12[64:128, si, 0:64] = gv(1, s, 2)
        wgr12[0:64, si, 64:128] = gv(2, s, 0)
        wgr12[64:128, si, 64:128] = gv(2, s, 2)
        wgr12b[:, si, 0:64] = gv(1, s, 1)
        wgr12b[:, si, 64:128] = gv(2, s, 1)
    out["wgr0"], out["wgr0b"] = _bf(mdup(wgr0)), _bf(mdup(wgr0b))
    out["wgr12"], out["wgr12b"] = _bf(wgr12), _bf(wgr12b)

    # E
    def ev(v, kd, kh):
        if kd in KDSETS[v]:
            return wrq[:, :, kd, kh, 2].T
        return np.zeros((CF, CF), np.float32)

    we0 = np.zeros((128, 3, CF), np.float32)
    we0b = np.zeros((64, 3, CF), np.float32)
    we12 = np.zeros((128, 3, 128), np.float32)
    we12b = np.zeros((64, 3, 128), np.float32)
    for kd in range(3):
        we0[0:64, kd, :] = ev(0, kd, 0)
        we0[64:128, kd, :] = ev(0, kd, 2)
        we0b[:, kd, :] = ev(0, kd, 1)
        we12[0:64, kd, 0:64] = ev(1, kd, 0)
        we12[64:128, kd, 0:64] = ev(1, kd, 2)
        we12[0:64, kd, 64:128] = ev(2, kd, 0)
        we12[64:128, kd, 64:128] = ev(2, kd, 2)
        we12b[:, kd, 0:64] = ev(1, kd, 1)
        we12b[:, kd, 64:128] = ev(2, kd, 1)
    out["we0"], out["we0b"] = _bf(mdup(we0)), _bf(mdup(we0b))
    out["we12"], out["we12b"] = _bf(we12), _bf(we12b)

    # F
    def fwf(fi, kh, kw):
        var, u, kws = F_COMBOS[fi]
        acc = np.zeros((CF, CF), np.float32)
        for kd in KDSETS[var]:
            if kd > u + kw:
                acc += wlq[:, :, kd, kh, kw]
        return acc.T

    wfq = np.zeros((128, NFQ, 128), np.float32)
    wfbq = np.zeros((64, NFQ, 128), np.float32)
    for q, (g, kw) in enumerate(FQ):
        fa, fb = FGROUPS[g]
        if fa is not None:
            wfq[0:64, q, 0:64] = fwf(fa, 0, kw)
            wfq[64:128, q, 0:64] = fwf(fa, 2, kw)
            wfbq[:, q, 0:64] = fwf(fa, 1, kw)
        if fb is not None:
            wfq[0:64, q, 64:128] = fwf(fb, 0, kw)
            wfq[64:128, q, 64:128] = fwf(fb, 2, kw)
            wfbq[:, q, 64:128] = fwf(fb, 1, kw)
    out["wf"], out["wfb"] = _bf(wfq), _bf(wfbq)

    # conv3b pair-packed
    wct = np.zeros((128, 9, 128), np.float32)
    wut = np.zeros((128, 9, 128), np.float32)
    for t in range(9):
        kh, kw = divmod(t, 3)
        wct[0:64, t, 0:64] = w3bq[:, :, 1, kh, kw].T
        wct[0:64, t, 64:128] = w3bq[:, :, 0, kh, kw].T
        wct[64:128, t, 0:64] = w3bq[:, :, 2, kh, kw].T
        wct[64:128, t, 64:128] = w3bq[:, :, 1, kh, kw].T
        wut[0:64, t, 0:64] = w3bq[:, :, 0, kh, kw].T
        wut[64:128, t, 64:128] = w3bq[:, :, 2, kh, kw].T
    out["wct"], out["wut"] = _bf(wct), _bf(wut)

    out["b1c"] = np.concatenate([b1, b1]).reshape(128, 1)
    out["b1r"] = _bf(np.concatenate([b1, b1]).reshape(1, 128))
    out["b3a2"] = np.concatenate([b3a, b3a]).reshape(128, 1)
    out["b3b2"] = np.concatenate([b3b, b3b]).reshape(128, 1)
    out["_b3a"] = b3a
    out["_b3b"] = b3b
    out["_w3b"] = w3bq
    return out


def _host_y_init(w3bq, b3a, b3b, c):
    """Const-filled initial y [D, CF, HLOC, W] for core c (f32)."""
    a0 = np.maximum(b3a, 0.0)
    y = np.zeros((D, CF, HLOC, W), np.float32)
    pre = np.einsum("oikhw,i->okhw", w3bq, a0)  # [o, kd, kh, kw]
    for r in range(HLOC):
        g = 6 * c + r
        khs = [kh for kh in range(3) if 0 <= g + kh - 1 < H]
        for d in range(D):
            wlo = max(0, d - 4)
            if wlo == 0:
                continue
            kds = [kd for kd in range(3) if 0 <= d + kd - 1 < D]
            cint = np.maximum(pre[:, kds][:, :, khs].sum((1, 2, 3)) + b3b, 0.0)
            cw0 = np.maximum(
                pre[:, kds][:, :, khs][:, :, :, 1:].sum((1, 2, 3)) + b3b, 0.0
            )
            y[d, :, r, 1:wlo] = cint[:, None]
            y[d, :, r, 0] = cw0
    return y


def _per_core_inputs(inputs, shared, c):
    r0 = 6 * c
    rows = np.arange(r0 - 2, r0 + 8)
    valid = (rows >= 0) & (rows < H)

    def slc(x):
        out = np.zeros((CIN, ROWS_IN, W), np.float32)
        out[:, valid] = x[0][:, rows[valid]]
        return _bf(out.reshape(CIN, ROWS_IN * W))

    m = {k: v for k, v in shared.items() if not k.startswith("_")}
    m["xl"] = slc(np.asarray(inputs["left_features"], np.float32))
    m["xr"] = slc(np.asarray(inputs["right_features"], np.float32))
    m["rowm"] = _bf(np.broadcast_to(valid.astype(np.float32), (128, ROWS_IN)))
    m["rmw"] = _bf(
        np.broadcast_to(valid.astype(np.float32)[None, :, None], (1, ROWS_IN, W))
    )
    arows = np.arange(r0 - 1, r0 + 7)
    gvals = np.where((arows >= 0) & (arows < H), 0.0, NEG).astype(np.float32)
    m["grm"] = np.broadcast_to(gvals, (128, ROWS_A)).copy()
    return m


_EXEC_CACHE = None


def _get_exec():
    """Build the SPMD executable once; reuse across kernel() calls."""
    global _EXEC_CACHE
    if _EXEC_CACHE is not None:
        return _EXEC_CACHE
    import jax
    import concourse.mybir as mb
    from concourse import bass2jax
    from jax.experimental.shard_map import shard_map
    from jax.sharding import Mesh, PartitionSpec

    nc = _get_nc()
    bass2jax.install_neuronx_cc_hook()
    partition_name = nc.partition_id_tensor.name if nc.partition_id_tensor else None
    in_names, out_names, out_avals = [], [], []
    for alloc in nc.m.functions[0].allocations:
        if not isinstance(alloc, mb.MemoryLocationSet):
            continue
        name = alloc.memorylocations[0].name
        if alloc.kind == "ExternalInput":
            if name != partition_name:
                in_names.append(name)
        elif alloc.kind == "ExternalOutput":
            shape = tuple(alloc.tensor_shape)
            dtype = mb.dt.np(alloc.dtype)
            out_names.append(name)
            out_avals.append(jax.core.ShapedArray(shape, dtype))
    n_params = len(in_names)
    all_in = list(in_names) + list(out_names)
    if partition_name is not None:
        all_in.append(partition_name)

    def _body(*args):
        operands = list(args)
        if partition_name is not None:
            operands.append(bass2jax.partition_id_tensor())
        outs = bass2jax._bass_exec_p.bind(
            *operands,
            out_avals=tuple(out_avals),
            in_names=tuple(all_in),
            out_names=tuple(out_names),
            lowering_input_output_aliases=(),
            sim_require_finite=True,
            sim_require_nnan=True,
            nc=nc,
        )
        return tuple(outs)

    devices = jax.devices()[:NC]
    mesh = Mesh(np.asarray(devices), ("core",))
    n_outs = len(out_names)
    sharded = jax.jit(
        shard_map(
            _body,
            mesh=mesh,
            in_specs=(PartitionSpec("core"),) * (n_params + n_outs),
            out_specs=(PartitionSpec("core"),) * n_outs,
            check_rep=False,
        ),
        donate_argnums=tuple(range(n_params, n_params + n_outs)),
        keep_unused=True,
    )
    _EXEC_CACHE = (sharded, in_names, out_names, out_avals)
    return _EXEC_CACHE


def _run(in_maps, out_inits):
    sharded, in_names, out_names, out_avals = _get_exec()
    concat_in = [
        np.concatenate([np.asarray(in_maps[c][nm]) for c in range(NC)], axis=0)
        for nm in in_names
    ]
    concat_outs = [
        np.concatenate([np.asarray(out_inits[c][nm]) for c in range(NC)], axis=0)
        for nm in out_names
    ]
    out_arrs = sharded(*concat_in, *concat_outs)
    return [
        {
            nm: np.asarray(out_arrs[i]).reshape(NC, *out_avals[i].shape)[c]
            for i, nm in enumerate(out_names)
        }
        for c in range(NC)
    ]


def kernel(**inputs):
    shared = _prep_weights(inputs)
    in_maps = [_per_core_inputs(inputs, shared, c) for c in range(NC)]
    out_inits = [
        {"y": _host_y_init(shared["_w3b"], shared["_b3a"], shared["_b3b"], c)}
        for c in range(NC)
    ]
    results = _run(in_maps, out_inits)
    full = np.zeros((CF, D, H, W), np.float32)
    for c in range(NC):
        y = results[c]["y"]  # [48, 64, 6, 160]
        full[:, :, 6 * c : 6 * c + 6, :] = y.transpose(1, 0, 2, 3)
    return full.reshape(1, CF * D, H, W)


# revision 27
# speedup vs baseline: 1.0634x; 1.0056x over previous
"""Trainium2 Bass kernel for nn_CostVolume: H-sharded across 8 NeuronCores.

v2: bf16 matmuls + M=128 pair-packing for conv3b + narrowed compute.

- BN folded into conv weights on host; all matmul operands bf16 (end-to-end
  rel err ~5e-3 vs fp32 reference, tolerance 2e-2).
- down(): 1x1 conv K=1024; x shipped bf16, loaded as 8 big DMAs per side.
- conv3a collapsed into G_L/G_R/E/F 2D convs (cost-volume shift structure);
  K-packed over (kh0,kh2) via row-shifted stacked tiles Lp2/Rp2, M-packed
  over variant pairs. b3a baked into G_L evac.
- A[d] assembled per-d by DVE (add/sub/relu) into paired tiles
  C_j = [A[2j]; A[2j+1]]; U_j = [A[2j-1]; A[2j+2]] built by 2 half-copies.
- conv3b M=128 pair-packed: outputs (2j, 2j+1) computed together, 18 matmuls
  per (pair, 3-row chunk) instead of 30: 9 taps on C_j ([w1;w0],[w2;w1]) and
  9 on U_j ([w0;0],[0;w2]).
- w < d-4 output region is constant (masked cost volume): skipped on-device,
  pre-filled host-side in the initial y buffer; A tiles rely on ring-stale
  columns matching the same constant.
"""

import sys

sys.path.insert(0, "/opt/trn_rl_repo")

import numpy as np
import ml_dtypes
import concourse.bass as bass
import concourse.bacc as bacc
import concourse.mybir as mybir
from concourse import tile

F32 = mybir.dt.float32
BF = mybir.dt.bfloat16
I32 = mybir.dt.int32
RELU = mybir.ActivationFunctionType.Relu
IDENT = mybir.ActivationFunctionType.Identity

H, W, D, CF, CIN = 48, 160, 48, 64, 1024
EPS = 1e-5
NC = 8
HLOC = 6
ROWS_IN = 10
ROWS_A = 8
WP = 162
GVLO = -10  # Gr col range [v=-10, 160)
GW = 170
NRING = 4
EV0 = 112
EW = 48
FW = 52
NEG = -1.0e30
S5 = (-2, -1, 0, 1, 2)

KDSETS = {0: (0, 1, 2), 1: (1, 2), 2: (0, 1)}


def _var(d):
    return 1 if d == 0 else (2 if d == D - 1 else 0)


def _fold_bn(w, b, g, beta, m, v):
    s = (g / np.sqrt(v + EPS)).astype(np.float32)
    return (w * s.reshape(-1, *([1] * (w.ndim - 1)))).astype(np.float32), (
        (b - m) * s + beta
    ).astype(np.float32)


def _f_combos():
    combos = []
    for var, kds in KDSETS.items():
        urange = (0, 1) if var == 1 else (-2, -1, 0, 1)
        for u in urange:
            kws = [kw for kw in range(3) if any(kd > u + kw for kd in kds)]
            if kws:
                combos.append((var, u, kws))
    return combos


F_COMBOS = _f_combos()
FIDX = {(var, u): fi for fi, (var, u, _) in enumerate(F_COMBOS)}
# M-pack groups for F: (top fi, bottom fi), kw list per group
FGROUPS = [(0, 1), (2, 7), (4, 6), (5, 3), (None, 8)]
FKWL = [[0, 1, 2], [0, 1], [0, 1, 2], [0], [0]]
FQ = [(g, kw) for g, kws in enumerate(FKWL) for kw in kws]  # 10 flat taps
NFQ = len(FQ)


def _bcast0(ap, n):
    return bass.AP(ap.tensor, ap.offset, list(ap.ap) + [[0, n]])


def _bf(x):
    return np.asarray(x, np.float32).astype(ml_dtypes.bfloat16)


def build_nc():
    nc = bacc.Bacc("TRN2", target_bir_lowering=False, debug=False, num_devices=NC)

    xl_d = nc.dram_tensor("xl", [CIN, ROWS_IN * W], BF, kind="ExternalInput")
    xr_d = nc.dram_tensor("xr", [CIN, ROWS_IN * W], BF, kind="ExternalInput")
    w1t_d = nc.dram_tensor("w1t", [128, 8, 128], BF, kind="ExternalInput")
    wgl0_d = nc.dram_tensor("wgl0", [128, 3, 128], BF, kind="ExternalInput")
    wgl0b_d = nc.dram_tensor("wgl0b", [64, 3, 128], BF, kind="ExternalInput")
    wgl12_d = nc.dram_tensor("wgl12", [128, 3, 128], BF, kind="ExternalInput")
    wgl12b_d = nc.dram_tensor("wgl12b", [64, 3, 128], BF, kind="ExternalInput")
    wgr0_d = nc.dram_tensor("wgr0", [128, 5, 128], BF, kind="ExternalInput")
    wgr0b_d = nc.dram_tensor("wgr0b", [64, 5, 128], BF, kind="ExternalInput")
    wgr12_d = nc.dram_tensor("wgr12", [128, 5, 128], BF, kind="ExternalInput")
    wgr12b_d = nc.dram_tensor("wgr12b", [64, 5, 128], BF, kind="ExternalInput")
    we0_d = nc.dram_tensor("we0", [128, 3, 128], BF, kind="ExternalInput")
    we0b_d = nc.dram_tensor("we0b", [64, 3, 128], BF, kind="ExternalInput")
    we12_d = nc.dram_tensor("we12", [128, 3, 128], BF, kind="ExternalInput")
    we12b_d = nc.dram_tensor("we12b", [64, 3, 128], BF, kind="ExternalInput")
    wf_d = nc.dram_tensor("wf", [128, NFQ, 128], BF, kind="ExternalInput")
    wfb_d = nc.dram_tensor("wfb", [64, NFQ, 128], BF, kind="ExternalInput")
    wct_d = nc.dram_tensor("wct", [128, 9, 128], BF, kind="ExternalInput")
    wut_d = nc.dram_tensor("wut", [128, 9, 128], BF, kind="ExternalInput")
    b1c_d = nc.dram_tensor("b1c", [128, 1], F32, kind="ExternalInput")
    b1r_d = nc.dram_tensor("b1r", [1, 128], BF, kind="ExternalInput")
    rmw_d = nc.dram_tensor("rmw", [1, ROWS_IN, W], BF, kind="ExternalInput")
    b3a2_d = nc.dram_tensor("b3a2", [128, 1], F32, kind="ExternalInput")
    b3b2_d = nc.dram_tensor("b3b2", [128, 1], F32, kind="ExternalInput")
    rowm_d = nc.dram_tensor("rowm", [128, ROWS_IN], BF, kind="ExternalInput")
    grm_d = nc.dram_tensor("grm", [128, ROWS_A], F32, kind="ExternalInput")
    y_d = nc.dram_tensor("y", [D, CF, HLOC, W], F32, kind="ExternalOutput")

    with tile.TileContext(nc) as tc:
        with (
            tc.tile_pool(name="wpool", bufs=1) as wpool,
            tc.tile_pool(name="big", bufs=1) as big,
            tc.tile_pool(name="ost", bufs=4) as ostp,
            tc.tile_pool(name="psd", bufs=2, space="PSUM") as psd_p,
            tc.tile_pool(name="psg", bufs=2, space="PSUM") as psg_p,
            tc.tile_pool(name="ps3", bufs=4, space="PSUM") as ps3_p,
        ):
            # ---- persistent tiles ----
            xt = [big.tile([128, 8, ROWS_IN, W], BF, name=f"xt{s}") for s in range(2)]

            # ---- input/weight DMAs, ordered for startup overlap ----
            def xload(side, x_d, r0, r1):
                # one DMA covering all 8 K-chunks for rows [r0, r1)
                nc.sync.dma_start(
                    xt[side][:, :, r0:r1, :],
                    x_d[:, W * r0 : W * r1].rearrange(
                        "(k p) (r c) -> p k r c", p=128, r=r1 - r0
                    ),
                )

            def wt(dram, shape):
                t = wpool.tile(shape, BF, name=dram.name + "_t")
                nc.sync.dma_start(t[:], dram[:])
                return t

            xload(0, xl_d, 0, 2)
            w1t = wt(w1t_d, [128, 8, 128])
            b1r = wt(b1r_d, [1, 128])
            rmw = wt(rmw_d, [1, ROWS_IN, W])
            for r0 in (2, 4, 6, 8):
                xload(0, xl_d, r0, r0 + 2)
            for r0 in (0, 2, 4, 6, 8):
                xload(1, xr_d, r0, r0 + 2)
            wgl0 = wt(wgl0_d, [128, 3, 128])
            wgl0b = wt(wgl0b_d, [64, 3, 128])
            wgl12 = wt(wgl12_d, [128, 3, 128])
            wgl12b = wt(wgl12b_d, [64, 3, 128])
            rowm = wpool.tile([128, ROWS_IN], BF)
            nc.sync.dma_start(rowm[:], rowm_d[:])
            b3a2 = wpool.tile([128, 1], F32)
            nc.sync.dma_start(b3a2[:], b3a2_d[:])
            wf = wt(wf_d, [128, NFQ, 128])
            wfb = wt(wfb_d, [64, NFQ, 128])
            wgr0 = wt(wgr0_d, [128, 5, 128])
            wgr0b = wt(wgr0b_d, [64, 5, 128])
            wgr12 = wt(wgr12_d, [128, 5, 128])
            wgr12b = wt(wgr12b_d, [64, 5, 128])
            grm = wpool.tile([128, ROWS_A], F32)
            nc.sync.dma_start(grm[:], grm_d[:])
            we0 = wt(we0_d, [128, 3, 128])
            we0b = wt(we0b_d, [64, 3, 128])
            we12 = wt(we12_d, [128, 3, 128])
            we12b = wt(we12b_d, [64, 3, 128])
            wct = wt(wct_d, [128, 9, 128])
            wut = wt(wut_d, [128, 9, 128])
            b3b2 = wpool.tile([128, 1], F32)
            nc.sync.dma_start(b3b2[:], b3b2_d[:])
            Lp2 = big.tile([128, ROWS_IN, WP], BF)
            Rp2 = big.tile([128, ROWS_IN, 212], BF)
            Gld = big.tile([128, ROWS_A, W], BF)  # var0 both halves
            Gle = big.tile([128, ROWS_A, W], BF)  # top var1, bottom var2
            Grd = big.tile([128, ROWS_A, GW], BF)
            Gre = big.tile([128, ROWS_A, GW], BF)
            Etd = big.tile([128, ROWS_A, EW], BF)
            Ete = big.tile([128, ROWS_A, EW], BF)
            Fts = [big.tile([128, ROWS_A, FW], BF, name=f"Ft{i}") for i in range(4)]
            FE = [big.tile([128, ROWS_A, FW], BF, name=f"FE{i}") for i in range(3)]
            Cring = [
                big.tile([128, ROWS_A, WP], BF, name=f"C{i}") for i in range(NRING)
            ]
            Uring = [
                big.tile([128, ROWS_A, WP], BF, name=f"U{i}") for i in range(NRING)
            ]

            # warm the ACT function table while DMAs stream
            scr = wpool.tile([1, 2], F32, name="scr")
            nc.vector.memset(scr[:], 0)
            nc.scalar.activation(scr[:], scr[:], RELU)

            # pad memsets (before writes)
            nc.vector.memset(Lp2[0:64, :, 0:1], 0)
            nc.vector.memset(Lp2[0:64, :, 161:162], 0)
            nc.vector.memset(Rp2[0:64, :, 0:50], 0)
            nc.vector.memset(Rp2[0:64, :, 210:212], 0)
            for t in Cring + Uring:
                nc.vector.memset(t[:, :, 0:1], 0)
                nc.vector.memset(t[:, :, 161:162], 0)
            nc.vector.memset(Uring[0][0:64, :, :].bitcast(I32), 0)

            # ---- phase 1: down() ----
            def phase1(side):
                tgt, c0 = (Lp2, 1) if side == 0 else (Rp2, 50)
                for c5 in range(5):
                    r = 2 * c5
                    ps = psd_p.tile([128, 2, W], F32, tag="psd")
                    for k in range(8):
                        nc.tensor.matmul(
                            ps[:],
                            w1t[:, k, :],
                            xt[side][:, k, r : r + 2, :],
                            start=(k == 0),
                            stop=False,
                        )
                    # bias masked per-row: psum += b1 (x) rowmask -> invalid
                    # rows stay exactly zero through relu
                    nc.tensor.matmul(
                        ps[:],
                        b1r[:],
                        rmw[:, r : r + 2, :],
                        start=False,
                        stop=True,
                    )
                    nc.scalar.activation(
                        tgt[0:64, r : r + 2, c0 : c0 + W], ps[0:64], RELU
                    )
                    if c5 > 0:
                        # bottom half holds rows shifted by 2 (kh2 K-packing)
                        nc.scalar.activation(
                            tgt[64:128, r - 2 : r, c0 : c0 + W], ps[64:128], RELU
                        )

            phase1(0)

            # ---- phase 2 ----
            # G_L: per 2-row chunk, var0 solo then (var1,var2) packed
            for r in (0, 2, 4, 6):
                ps = psg_p.tile([128, 2, W], F32, tag="psg")
                for kw in range(3):
                    nc.tensor.matmul(
                        ps[:],
                        wgl0[:, kw, :],
                        Lp2[:, r : r + 2, kw : kw + W],
                        start=(kw == 0),
                        stop=False,
                    )
                for kw in range(3):
                    nc.tensor.matmul(
                        ps[:],
                        wgl0b[:, kw, :],
                        Lp2[0:64, r + 1 : r + 3, kw : kw + W],
                        start=False,
                        stop=(kw == 2),
                    )
                nc.scalar.activation(Gld[:, r : r + 2, :], ps[:], IDENT, bias=b3a2[:])
                ps2 = psg_p.tile([128, 2, W], F32, tag="psg")
                for kw in range(3):
                    nc.tensor.matmul(
                        ps2[:],
                        wgl12[:, kw, :],
                        Lp2[:, r : r + 2, kw : kw + W],
                        start=(kw == 0),
                        stop=False,
                    )
                for kw in range(3):
                    nc.tensor.matmul(
                        ps2[:],
                        wgl12b[:, kw, :],
                        Lp2[0:64, r + 1 : r + 3, kw : kw + W],
                        start=False,
                        stop=(kw == 2),
                    )
                nc.scalar.activation(Gle[:, r : r + 2, :], ps2[:], IDENT, bias=b3a2[:])

            # F groups
            for g, (fa, fb) in enumerate(FGROUPS):
                qs = [q for q, (gg, _) in enumerate(FQ) if gg == g]
                ps = psg_p.tile([128, ROWS_A, FW], F32, tag="psg")
                n = 2 * len(qs)
                i = 0
                for q in qs:
                    kw = FQ[q][1]
                    nc.tensor.matmul(
                        ps[:],
                        wf[:, q, :],
                        Lp2[:, 0:ROWS_A, kw : kw + FW],
                        start=(i == 0),
                        stop=False,
                    )
                    i += 1
                for q in qs:
                    kw = FQ[q][1]
                    nc.tensor.matmul(
                        ps[:],
                        wfb[:, q, :],
                        Lp2[0:64, 1 : 1 + ROWS_A, kw : kw + FW],
                        start=False,
                        stop=(i == n - 1),
                    )
                    i += 1
                if g == 0:
                    nc.scalar.activation(Fts[0][0:64], ps[0:64], IDENT)
                    nc.scalar.activation(Fts[1][64:128], ps[64:128], IDENT)
                elif g == 1:
                    nc.scalar.activation(Fts[2][0:64], ps[0:64], IDENT)
                    nc.scalar.activation(FE[1][64:128], ps[64:128], IDENT)
                elif g == 2:
                    nc.scalar.activation(FE[0][:], ps[:], IDENT)
                elif g == 3:
                    nc.scalar.activation(FE[1][0:64], ps[0:64], IDENT)
                    nc.scalar.activation(Fts[3][64:128], ps[64:128], IDENT)
                else:
                    nc.scalar.activation(FE[2][64:128], ps[64:128], IDENT)
            nc.scalar.dma_start(Fts[0][64:128, :, :], Fts[0][0:64, :, :])
            nc.scalar.dma_start(Fts[1][0:64, :, :], Fts[1][64:128, :, :])
            nc.scalar.dma_start(Fts[2][64:128, :, :], Fts[2][0:64, :, :])
            nc.scalar.dma_start(Fts[3][0:64, :, :], Fts[3][64:128, :, :])

            phase1(1)

            # E
            for r in (0, 2, 4, 6):
                ps = psg_p.tile([128, 2, GW], F32, tag="psg")
                for si, s in enumerate(S5):
                    nc.tensor.matmul(
                        ps[:],
                        wgr0[:, si, :],
                        Rp2[:, r : r + 2, 40 + s : 40 + s + GW],
                        start=(si == 0),
                        stop=False,
                    )
                for si, s in enumerate(S5):
                    nc.tensor.matmul(
                        ps[:],
                        wgr0b[:, si, :],
                        Rp2[0:64, r + 1 : r + 3, 40 + s : 40 + s + GW],
                        start=False,
                        stop=(si == 4),
                    )
                nc.vector.tensor_add(
                    Grd[:, r : r + 2, :], ps[:], _bcast0(grm[:, r : r + 2], GW)
                )
                ps2 = psg_p.tile([128, 2, GW], F32, tag="psg")
                for si, s in enumerate(S5):
                    nc.tensor.matmul(
                        ps2[:],
                        wgr12[:, si, :],
                        Rp2[:, r : r + 2, 40 + s : 40 + s + GW],
                        start=(si == 0),
                        stop=False,
                    )
                for si, s in enumerate(S5):
                    nc.tensor.matmul(
                        ps2[:],
                        wgr12b[:, si, :],
                        Rp2[0:64, r + 1 : r + 3, 40 + s : 40 + s + GW],
                        start=False,
                        stop=(si == 4),
                    )
                nc.vector.tensor_add(
                    Gre[:, r : r + 2, :], ps2[:], _bcast0(grm[:, r : r + 2], GW)
                )

            # E
            psE = psg_p.tile([128, ROWS_A, EW], F32, tag="psg")
            for kd in range(3):
                nc.tensor.matmul(
                    psE[:],
                    we0[:, kd, :],
                    Rp2[:, 0:ROWS_A, 164 - kd : 164 - kd + EW],
                    start=(kd == 0),
                    stop=False,
                )
            for kd in range(3):
                nc.tensor.matmul(
                    psE[:],
                    we0b[:, kd, :],
                    Rp2[0:64, 1 : 1 + ROWS_A, 164 - kd : 164 - kd + EW],
                    start=False,
                    stop=(kd == 2),
                )
            nc.scalar.activation(Etd[:], psE[:], IDENT)
            psE2 = psg_p.tile([128, ROWS_A, EW], F32, tag="psg")
            for kd in range(3):
                nc.tensor.matmul(
                    psE2[:],
                    we12[:, kd, :],
                    Rp2[:, 0:ROWS_A, 164 - kd : 164 - kd + EW],
                    start=(kd == 0),
                    stop=False,
                )
            for kd in range(3):
                nc.tensor.matmul(
                    psE2[:],
                    we12b[:, kd, :],
                    Rp2[0:64, 1 : 1 + ROWS_A, 164 - kd : 164 - kd + EW],
                    start=False,
                    stop=(kd == 2),
                )
            nc.scalar.activation(Ete[:], psE2[:], IDENT)

            # band-sub tile lookup (var, u) -> tile (slice by C half)
            BANDT = {
                (0, -2): Fts[0], (0, -1): Fts[1], (0, 0): Fts[2], (0, 1): Fts[3],
                (1, 0): FE[0], (1, 1): FE[1],
                (2, -2): FE[0], (2, -1): FE[1], (2, 0): FE[2],
            }

            # ---- d-loop ----
            # C-matmuls of pair p run at iter p+1, U-matmuls + evac at p+2:
            # every PE input is produced at least one iteration earlier.
            ps_open = {}
            for it in range(26):
                if it == 22:
                    # U_23.bottom = A[48] = 0 (after U_19's last read at it=21)
                    nc.vector.memset(Uring[23 % NRING][64:128, :, :].bitcast(I32), 0)
                if it < 24:
                    j = it
                    Cj = Cring[j % NRING]
                    for d in (2 * j, 2 * j + 1):
                        half = d % 2
                        sl = slice(0, 64) if half == 0 else slice(64, 128)
                        var = _var(d)
                        Glt = Gld if var == 0 else Gle
                        Grt = Grd if var == 0 else Gre
                        Ett = Etd if var == 0 else Ete
                        alo = max(0, d - 10)
                        blo = max(0, d - 2)
                        # region1 first: keeps the ACT queue ahead of DVE
                        if blo > alo:
                            nc.scalar.activation(
                                Cj[sl, :, 1 + alo : 1 + blo],
                                Grt[sl, :, alo - d - GVLO : blo - d - GVLO],
                                RELU,
                                bias=b3a2[sl],
                            )
                        # region2: [blo, W)
                        nc.vector.tensor_add(
                            Cj[sl, :, 1 + blo : 161],
                            Glt[sl, :, blo:W],
                            Grt[sl, :, blo - d - GVLO : W - d - GVLO],
                        )
                        for u in (-2, -1, 0, 1):
                            w = d + u
                            ft = BANDT.get((var, u))
                            if ft is not None and 0 <= w < W:
                                nc.vector.tensor_sub(
                                    Cj[sl, :, 1 + w : 2 + w],
                                    Cj[sl, :, 1 + w : 2 + w],
                                    ft[sl, :, w : w + 1],
                                )
                        nc.vector.tensor_sub(
                            Cj[sl, :, 160:161],
                            Cj[sl, :, 160:161],
                            Ett[sl, :, 47 - d : 48 - d],
                        )
                        nc.vector.tensor_scalar_max(
                            Cj[sl, :, 1 + blo : 161], Cj[sl, :, 1 + blo : 161], 0.0
                        )
                    if j > 0:
                        nc.sync.dma_start(
                            Uring[j % NRING][0:64, :, :],
                            Cring[(j - 1) % NRING][64:128, :, :],
                        )
                        nc.sync.dma_start(
                            Uring[(j - 1) % NRING][64:128, :, :], Cj[0:64, :, :]
                        )
                for p in ([it - 2] if it >= 2 else []):
                    d0 = 2 * p
                    wlo = max(0, d0 - 4)
                    width = W - wlo
                    Up = Uring[p % NRING]
                    for j0 in (0, 3):
                        ps = ps_open.pop((p, j0))
                        for t in range(9):
                            kh, kw = divmod(t, 3)
                            nc.tensor.matmul(
                                ps[:, :, 0:width],
                                wut[:, t, :],
                                Up[:, j0 + kh : j0 + kh + 3, wlo + kw : wlo + kw + width],
                                start=False,
                                stop=(t == 8),
                            )
                        ost = ostp.tile([128, 3, W], F32, tag="ost")
                        nc.scalar.activation(
                            ost[:, :, 0:width], ps[:, :, 0:width], RELU, bias=b3b2[:]
                        )
                        nc.sync.dma_start(
                            y_d[d0 : d0 + 2, :, j0 : j0 + 3, wlo:W],
                            ost[:, :, 0:width],
                        )
                if 1 <= it <= 24:
                    p = it - 1
                    wlo = max(0, 2 * p - 4)
                    width = W - wlo
                    Cp = Cring[p % NRING]
                    for j0 in (0, 3):
                        ps = ps3_p.tile([128, 3, W], F32, tag="ps3")
                        ps_open[(p, j0)] = ps
                        for t in range(9):
                            kh, kw = divmod(t, 3)
                            nc.tensor.matmul(
                                ps[:, :, 0:width],
                                wct[:, t, :],
                                Cp[:, j0 + kh : j0 + kh + 3, wlo + kw : wlo + kw + width],
                                start=(t == 0),
                                stop=False,
                            )

    nc.finalize()
    return nc


_NC_CACHE = None


def _get_nc():
    global _NC_CACHE
    if _NC_CACHE is None:
        _NC_CACHE = build_nc()
    return _NC_CACHE


def _prep_weights(inputs):
    w1, b1 = _fold_bn(
        inputs["conv1_w"], inputs["conv1_b"], inputs["bn1_g"], inputs["bn1_b"],
        inputs["bn1_m"], inputs["bn1_v"],
    )
    w3a, b3a = _fold_bn(
        inputs["c3a_w"], inputs["c3a_b"], inputs["bn3a_g"], inputs["bn3a_b"],
        inputs["bn3a_m"], inputs["bn3a_v"],
    )
    w3b, b3b = _fold_bn(
        inputs["c3b_w"], inputs["c3b_b"], inputs["bn3b_g"], inputs["bn3b_b"],
        inputs["bn3b_m"], inputs["bn3b_v"],
    )
    wl, wr = w3a[:, :CF], w3a[:, CF:]
    wlq = _bf(wl).astype(np.float32)
    wrq = _bf(wr).astype(np.float32)
    w3bq = _bf(w3b).astype(np.float32)

    def mdup(a):
        """Duplicate M columns: [..., 64] -> [..., 128] with both halves equal."""
        return np.concatenate([a, a], axis=-1)

    out = {}
    out["w1t"] = _bf(
        mdup(np.ascontiguousarray(w1.T.reshape(8, 128, CF).transpose(1, 0, 2)))
    )

    # G_L
    kv = {
        v: sum(wlq[:, :, kd] for kd in kds) for v, kds in KDSETS.items()
    }  # [o,i,kh,kw]
    wgl0 = np.zeros((128, 3, CF), np.float32)
    wgl0b = np.zeros((64, 3, CF), np.float32)
    # duplicated to [*, 3, 128] after fill (see below)
    wgl12 = np.zeros((128, 3, 128), np.float32)
    wgl12b = np.zeros((64, 3, 128), np.float32)
    for kw in range(3):
        wgl0[0:64, kw, :] = kv[0][:, :, 0, kw].T
        wgl0[64:128, kw, :] = kv[0][:, :, 2, kw].T
        wgl0b[:, kw, :] = kv[0][:, :, 1, kw].T
        wgl12[0:64, kw, 0:64] = kv[1][:, :, 0, kw].T
        wgl12[64:128, kw, 0:64] = kv[1][:, :, 2, kw].T
        wgl12[0:64, kw, 64:128] = kv[2][:, :, 0, kw].T
        wgl12[64:128, kw, 64:128] = kv[2][:, :, 2, kw].T
        wgl12b[:, kw, 0:64] = kv[1][:, :, 1, kw].T
        wgl12b[:, kw, 64:128] = kv[2][:, :, 1, kw].T
    out["wgl0"], out["wgl0b"] = _bf(mdup(wgl0)), _bf(mdup(wgl0b))
    out["wgl12"], out["wgl12b"] = _bf(wgl12), _bf(wgl12b)

    # G_R
    def gv(v, s, kh):
        acc = np.zeros((CF, CF), np.float32)
        for kd in KDSETS[v]:
            kw = s + kd
            if 0 <= kw < 3:
                acc += wrq[:, :, kd, kh, kw]
        return acc.T  # [i, o]

    wgr0 = np.zeros((128, 5, CF), np.float32)
    wgr0b = np.zeros((64, 5, CF), np.float32)
    wgr12 = np.zeros((128, 5, 128), np.float32)
    wgr12b = np.zeros((64, 5, 128), np.float32)
    for si, s in enumerate(S5):
        wgr0[0:64, si, :] = gv(0, s, 0)
        wgr0[64:128, si, :] = gv(0, s, 2)
        wgr0b[:, si, :] = gv(0, s, 1)
        wgr12[0:64, si, 0:64] = gv(1, s, 0)
        wgr12[64:128, si, 0:64] = gv(1, s, 2)
        wgr12[0:64, si, 64:128] = gv(2, s, 0)
        wgr12[64:128, si, 64:128] = gv(2, s, 2)
        wgr12b[:, si, 0:64] = gv(1, s, 1)
        wgr12b[:, si, 64:128] = gv(2, s, 1)
    out["wgr0"], out["wgr0b"] = _bf(mdup(wgr0)), _bf(mdup(wgr0b))
    out["wgr12"], out["wgr12b"] = _bf(wgr12), _bf(wgr12b)

    # E
    def ev(v, kd, kh):
        if kd in KDSETS[v]:
            return wrq[:, :, kd, kh, 2].T
        return np.zeros((CF, CF), np.float32)

    we0 = np.zeros((128, 3, CF), np.float32)
    we0b = np.zeros((64, 3, CF), np.float32)
    we12 = np.zeros((128, 3, 128), np.float32)
    we12b = np.zeros((64, 3, 128), np.float32)
    for kd in range(3):
        we0[0:64, kd, :] = ev(0, kd, 0)
        we0[64:128, kd, :] = ev(0, kd, 2)
        we0b[:, kd, :] = ev(0, kd, 1)
        we12[0:64, kd, 0:64] = ev(1, kd, 0)
        we12[64:128, kd, 0:64] = ev(1, kd, 2)
        we12[0:64, kd, 64:128] = ev(2, kd, 0)
        we12[64:128, kd, 64:128] = ev(2, kd, 2)
        we12b[:, kd, 0:64] = ev(1, kd, 1)
        we12b[:, kd, 64:128] = ev(2, kd, 1)
    out["we0"], out["we0b"] = _bf(mdup(we0)), _bf(mdup(we0b))
    out["we12"], out["we12b"] = _bf(we12), _bf(we12b)

    # F
    def fwf(fi, kh, kw):
        var, u, kws = F_COMBOS[fi]
        acc = np.zeros((CF, CF), np.float32)
        for kd in KDSETS[var]:
            if kd > u + kw:
                acc += wlq[:, :, kd, kh, kw]
        return acc.T

    wfq = np.zeros((128, NFQ, 128), np.float32)
    wfbq = np.zeros((64, NFQ, 128), np.float32)
    for q, (g, kw) in enumerate(FQ):
        fa, fb = FGROUPS[g]
        if fa is not None:
            wfq[0:64, q, 0:64] = fwf(fa, 0, kw)
            wfq[64:128, q, 0:64] = fwf(fa, 2, kw)
            wfbq[:, q, 0:64] = fwf(fa, 1, kw)
        if fb is not None:
            wfq[0:64, q, 64:128] = fwf(fb, 0, kw)
            wfq[64:128, q, 64:128] = fwf(fb, 2, kw)
            wfbq[:, q, 64:128] = fwf(fb, 1, kw)
    out["wf"], out["wfb"] = _bf(wfq), _bf(wfbq)

    # conv3b pair-packed
    wct = np.zeros((128, 9, 128), np.float32)
    wut = np.zeros((128, 9, 128), np.float32)
    for t in range(9):
        kh, kw = divmod(t, 3)
        wct[0:64, t, 0:64] = w3bq[:, :, 1, kh, kw].T
        wct[0:64, t, 64:128] = w3bq[:, :, 0, kh, kw].T
        wct[64:128, t, 0:64] = w3bq[:, :, 2, kh, kw].T
        wct[64:128, t, 64:128] = w3bq[:, :, 1, kh, kw].T
        wut[0:64, t, 0:64] = w3bq[:, :, 0, kh, kw].T
        wut[64:128, t, 64:128] = w3bq[:, :, 2, kh, kw].T
    out["wct"], out["wut"] = _bf(wct), _bf(wut)

    out["b1c"] = np.concatenate([b1, b1]).reshape(128, 1)
    out["b1r"] = _bf(np.concatenate([b1, b1]).reshape(1, 128))
    out["b3a2"] = np.concatenate([b3a, b3a]).reshape(128, 1)
    out["b3b2"] = np.concatenate([b3b, b3b]).reshape(128, 1)
    out["_b3a"] = b3a
    out["_b3b"] = b3b
    out["_w3b"] = w3bq
    return out


def _host_y_init(w3bq, b3a, b3b, c):
    """Const-filled initial y [D, CF, HLOC, W] for core c (f32)."""
    a0 = np.maximum(b3a, 0.0)
    y = np.zeros((D, CF, HLOC, W), np.float32)
    pre = np.einsum("oikhw,i->okhw", w3bq, a0)  # [o, kd, kh, kw]
    for r in range(HLOC):
        g = 6 * c + r
        khs = [kh for kh in range(3) if 0 <= g + kh - 1 < H]
        for d in range(D):
            wlo = max(0, d - 4)
            if wlo == 0:
                continue
            kds = [kd for kd in range(3) if 0 <= d + kd - 1 < D]
            cint = np.maximum(pre[:, kds][:, :, khs].sum((1, 2, 3)) + b3b, 0.0)
            cw0 = np.maximum(
                pre[:, kds][:, :, khs][:, :, :, 1:].sum((1, 2, 3)) + b3b, 0.0
            )
            y[d, :, r, 1:wlo] = cint[:, None]
            y[d, :, r, 0] = cw0
    return y


def _per_core_inputs(inputs, shared, c):
    r0 = 6 * c
    rows = np.arange(r0 - 2, r0 + 8)
    valid = (rows >= 0) & (rows < H)

    def slc(x):
        out = np.zeros((CIN, ROWS_IN, W), np.float32)
        out[:, valid] = x[0][:, rows[valid]]
        return _bf(out.reshape(CIN, ROWS_IN * W))

    m = {k: v for k, v in shared.items() if not k.startswith("_")}
    m["xl"] = slc(np.asarray(inputs["left_features"], np.float32))
    m["xr"] = slc(np.asarray(inputs["right_features"], np.float32))
    m["rowm"] = _bf(np.broadcast_to(valid.astype(np.float32), (128, ROWS_IN)))
    m["rmw"] = _bf(
        np.broadcast_to(valid.astype(np.float32)[None, :, None], (1, ROWS_IN, W))
    )
    arows = np.arange(r0 - 1, r0 + 7)
    gvals = np.where((arows >= 0) & (arows < H), 0.0, NEG).astype(np.float32)
    m["grm"] = np.broadcast_to(gvals, (128, ROWS_A)).copy()
    return m


_EXEC_CACHE = None


def _get_exec():
    """Build the SPMD executable once; reuse across kernel() calls."""
    global _EXEC_CACHE
    if _EXEC_CACHE is not None:
        return _EXEC_CACHE
    import jax
    import concourse.mybir as mb
    from concourse import bass2jax
    from jax.experimental.shard_map import shard_map
    from jax.sharding import Mesh, PartitionSpec

    nc = _get_nc()
    bass2jax.install_neuronx_cc_hook()
    partition_name = nc.partition_id_tensor.name if nc.partition_id_tensor else None
    in_names, out_names, out_avals = [], [], []
    for alloc in nc.m.functions[0].allocations:
        if not isinstance(alloc, mb.MemoryLocationSet):
            continue
        name = alloc.memorylocations[0].name
        if alloc.kind == "ExternalInput":
            if name != partition_name:
                in_names.append(name)
        elif alloc.kind == "ExternalOutput":
            shape = tuple(alloc.tensor_shape)
            dtype = mb.dt.np(alloc.dtype)
            out_names.append(name)
            out_avals.append(jax.core.ShapedArray(shape, dtype))
    n_params = len(in_names)
    all_in = list(in_names) + list(out_names)
    if partition_name is not None:
        all_in.append(partition_name)

    def _body(*args):
        operands = list(args)
        if partition_name is not None:
            operands.append(bass2jax.partition_id_tensor())
        outs = bass2jax._bass_exec_p.bind(
            *operands,
            out_avals=tuple(out_avals),
            in_names=tuple(all_in),
            out_names=tuple(out_names),
            lowering_input_output_aliases=(),
            sim_require_finite=True,
            sim_require_nnan=True,
            nc=nc,
        )
        return tuple(outs)

    devices = jax.devices()[:NC]
    mesh = Mesh(np.asarray(devices), ("core",))
    n_outs = len(out_names)
    sharded = jax.jit(
        shard_map(
            _body,
            mesh=mesh,
            in_specs=(PartitionSpec("core"),) * (n_params + n_outs),
            out_specs=(PartitionSpec("core"),) * n_outs,
            check_rep=False,
        ),
        donate_argnums=tuple(range(n_params, n_params + n_outs)),
        keep_unused=True,
    )
    _EXEC_CACHE = (sharded, in_names, out_names, out_avals)
    return _EXEC_CACHE


def _run(in_maps, out_inits):
    sharded, in_names, out_names, out_avals = _get_exec()
    concat_in = [
        np.concatenate([np.asarray(in_maps[c][nm]) for c in range(NC)], axis=0)
        for nm in in_names
    ]
    concat_outs = [
        np.concatenate([np.asarray(out_inits[c][nm]) for c in range(NC)], axis=0)
        for nm in out_names
    ]
    out_arrs = sharded(*concat_in, *concat_outs)
    return [
        {
            nm: np.asarray(out_arrs[i]).reshape(NC, *out_avals[i].shape)[c]
            for i, nm in enumerate(out_names)
        }
        for c in range(NC)
    ]


def kernel(**inputs):
    shared = _prep_weights(inputs)
    in_maps = [_per_core_inputs(inputs, shared, c) for c in range(NC)]
    out_inits = [
        {"y": _host_y_init(shared["_w3b"], shared["_b3a"], shared["_b3b"], c)}
        for c in range(NC)
    ]
    results = _run(in_maps, out_inits)
    full = np.zeros((CF, D, H, W), np.float32)
    for c in range(NC):
        y = results[c]["y"]  # [48, 64, 6, 160]
        full[:, :, 6 * c : 6 * c + 6, :] = y.transpose(1, 0, 2, 3)
    return full.reshape(1, CF * D, H, W)
